# revision 1
# baseline (speedup 1.0000x reference)
"""Trainium2 Bass kernel for nn_Attention (dense transformer block-attention).

Reference semantics (faithful reshape WITHOUT head transpose):
  qkv = x @ w_qkv                    # [B, N, 3*1024]
  q = qkv[..., 0:1024].reshape(B, 16, 2048, 64)   # head h <- token rows [h*128,(h+1)*128)
  out[b, n, c] = O_head(n//128)[(n%128)*16 + c//64, c%64]

Sharding: 32 (b, head) pairs over 8 cores -> each core: 1 batch x 4 heads.
Pure data parallel, no collectives. Host preps xT (bf16) per core + full w (bf16).

Layout tricks:
- Sub-token permutation n2' = cb*128 + r (softmax is permutation-invariant
  over keys; queries un-permuted via the output index mapping).
- qT/kT hold the 64-wide head dim DUPLICATED on both partition halves, so
  S matmuls contract K=128 (computing 2*q.k; factor folded into exp scale)
  and the layout transposes are clean [128,128] PE transposes.
- PV: out^T = [v|ones].T @ exp(S^T): softmax denominators ride in row 64.
- One PSUM layout all kernel long: tag ps = 2x[128,1024] (4 banks) used by
  projection accumulators / S ping-pong / tail transposes, tag po =
  1x[65,2048] (4 banks) for PV accumulators. No phase barriers.
"""

import numpy as np
import ml_dtypes

B, N, D = 2, 2048, 1024
H_PER_CORE = 4          # head-blocks per core
ROWS = 128              # token rows per head-block
SUB = 2048              # sub-tokens per head (128 rows * 16 col-blocks)
DH = 64                 # head dim
CB = 16                 # col-blocks per row
SCALE = 0.125           # 64 ** -0.5
N_CORES = 8

_GRAPH = None


def build_graph():
    global _GRAPH
    if _GRAPH is not None:
        return _GRAPH

    import concourse.mybir as mybir
    import concourse.tile as tile
    from concourse import bacc
    from concourse.masks import make_identity
    from contextlib import ExitStack

    f32 = mybir.dt.float32
    bf16 = mybir.dt.bfloat16
    EXP = mybir.ActivationFunctionType.Exp

    nc = bacc.Bacc("TRN2", target_bir_lowering=False, debug=False,
                   num_devices=N_CORES)

    xt_dram = nc.dram_tensor("xt", [D, H_PER_CORE * ROWS], bf16,
                             kind="ExternalInput")
    w_dram = nc.dram_tensor("w", [D, 3 * D], bf16, kind="ExternalInput")
    out_dram = nc.dram_tensor("out", [H_PER_CORE * ROWS, D], f32,
                              kind="ExternalOutput")

    KO = D // 128  # 8 k-tiles

    with tile.TileContext(nc) as tc, ExitStack() as ctx:
        const_pool = ctx.enter_context(tc.tile_pool(name="const", bufs=1))
        in_pool = ctx.enter_context(tc.tile_pool(name="inputs", bufs=1))
        qk_pool = ctx.enter_context(tc.tile_pool(name="qk", bufs=4))
        head_pool = ctx.enter_context(tc.tile_pool(name="head", bufs=1))
        pt_pool = ctx.enter_context(tc.tile_pool(name="pt", bufs=4))
        ot_pool = ctx.enter_context(tc.tile_pool(name="ot", bufs=3))
        small_pool = ctx.enter_context(tc.tile_pool(name="small", bufs=16))
        trt_pool = ctx.enter_context(tc.tile_pool(name="trt", bufs=16))
        psum = ctx.enter_context(tc.tile_pool(name="psum", bufs=2,
                                              space="PSUM"))
        psum2 = ctx.enter_context(tc.tile_pool(name="psum2", bufs=2,
                                               space="PSUM"))
        opsum = ctx.enter_context(tc.tile_pool(name="opsum", bufs=1,
                                               space="PSUM"))

        # ---- constants ----
        ident = const_pool.tile([128, 128], f32, tag="ident")
        make_identity(nc, ident[:])
        ident_bf = const_pool.tile([128, 128], bf16, tag="ident_bf")
        make_identity(nc, ident_bf[:])
        # warm up the exp table while the projection runs
        warm = const_pool.tile([128, 1], f32, tag="warm")
        nc.vector.memset(warm[:], 0.0)
        nc.scalar.activation(warm[:], warm[:], EXP)

        # ---- input DMA in first-consumption order ----
        xt_sbuf = in_pool.tile([128, KO, H_PER_CORE * ROWS], bf16, tag="xt")
        w_sbuf = in_pool.tile([128, KO, 3 * D], bf16, tag="w")
        nc.sync.dma_start(xt_sbuf[:, 0, 0:ROWS],
                          xt_dram.ap()[0:128, 0:ROWS])
        nc.sync.dma_start(w_sbuf[:, 0, 0:512], w_dram.ap()[0:128, 0:512])
        nc.sync.dma_start(xt_sbuf[:, 0, ROWS:],
                          xt_dram.ap()[0:128, ROWS:])
        nc.sync.dma_start(w_sbuf[:, 0, 512:1024],
                          w_dram.ap()[0:128, 512:1024])
        for ko in range(1, KO):
            nc.sync.dma_start(xt_sbuf[:, ko, :],
                              xt_dram.ap()[ko * 128:(ko + 1) * 128, :])
            nc.sync.dma_start(
                w_sbuf[:, ko, 0:1024],
                w_dram.ap()[ko * 128:(ko + 1) * 128, 0:1024])
        for half in range(1, 3):
            for ko in range(KO):
                nc.sync.dma_start(
                    w_sbuf[:, ko, half * 1024:(half + 1) * 1024],
                    w_dram.ap()[ko * 128:(ko + 1) * 128,
                                half * 1024:(half + 1) * 1024])

        # persistent per-head tiles (qT/kT carry duplicated d-halves)
        qT = [head_pool.tile([128, SUB], bf16, tag=f"qT{t}", name=f"qT{t}")
              for t in range(H_PER_CORE)]
        kT = [head_pool.tile([128, SUB], bf16, tag=f"kT{t}", name=f"kT{t}")
              for t in range(H_PER_CORE)]
        v_ones = [head_pool.tile([128, CB, DH + 1], bf16, tag=f"vo{t}",
                                 name=f"vo{t}")
                  for t in range(H_PER_CORE)]
        for t in range(H_PER_CORE):
            nc.vector.memset(v_ones[t][:, :, DH], 1.0)

        # ---- phase 1: projection (per block) ----
        qk2s = [None] * H_PER_CORE

        def emit_proj(t):
            qk2 = qk_pool.tile([128, 2 * CB, 128], bf16, tag="qk2",
                               name=f"qk2_{t}")
            qk2s[t] = qk2
            # six [128,512] accumulators from the small psum pool so the
            # attention pipeline (ps ping-pong + po) is never contended
            for ncx in range(6):
                ps = psum2.tile([128, 512], f32, tag="ps2")
                for ko in range(KO):
                    nc.tensor.matmul(
                        ps[:],
                        xt_sbuf[:, ko, t * ROWS:(t + 1) * ROWS],
                        w_sbuf[:, ko, ncx * 512:(ncx + 1) * 512],
                        start=(ko == 0), stop=(ko == KO - 1))
                src = ps[:].rearrange("p (a b) -> p a b", b=DH)
                if ncx < 4:
                    nc.vector.tensor_copy(
                        qk2[:, ncx * 8:(ncx + 1) * 8, 0:DH], src)
                    nc.vector.tensor_copy(
                        qk2[:, ncx * 8:(ncx + 1) * 8, DH:128], src)
                else:
                    nc.vector.tensor_copy(
                        v_ones[t][:, (ncx - 4) * 8:(ncx - 3) * 8, 0:DH],
                        src)

        def emit_transposes(t):
            if False:
                # XBAR DMA-transpose on the Sync engine: runs ahead of
                # need, overlapping earlier attention on PE/ACT.
                for cb in range(2 * CB):
                    dst = qT[t] if cb < CB else kT[t]
                    nc.sync.dma_start_transpose(
                        dst[:, (cb % CB) * 128:((cb % CB) + 1) * 128],
                        qk2s[t][:, cb, :])
            else:
                # head 0 gates the first attention: PE transposes (fast)
                for cb in range(2 * CB):
                    pst = psum2.tile([128, 128], bf16, tag="ps2")
                    nc.tensor.transpose(pst[:], qk2s[t][:, cb, :],
                                        ident_bf[:])
                    dst = qT[t] if cb < CB else kT[t]
                    nc.vector.tensor_copy(
                        dst[:, (cb % CB) * 128:((cb % CB) + 1) * 128],
                        pst[:])

        def emit_pass(t, ihalf, OTt):
            po = opsum.tile([DH + 1, SUB // 2], f32, tag="po")
            for j in range(CB):
                ps = psum.tile([128, 1024], f32, tag="ps")
                for sub in range(2):
                    ic = ihalf * 2 + sub
                    nc.tensor.matmul(
                        ps[:, sub * 512:(sub + 1) * 512],
                        kT[t][:, j * 128:(j + 1) * 128],
                        qT[t][:, ic * 512:(ic + 1) * 512],
                        start=True, stop=True)
                pt = pt_pool.tile([128, 1024], bf16, tag="pt")
                # psum holds 2*(q.k) (duplicated halves) -> scale/2
                nc.scalar.activation(pt[:], ps[:], EXP, scale=SCALE / 2)
                for sub in range(2):
                    nc.tensor.matmul(
                        po[:, sub * 512:(sub + 1) * 512],
                        v_ones[t][:, j, :],
                        pt[:, sub * 512:(sub + 1) * 512],
                        start=(j == 0), stop=(j == CB - 1))
            # evacuate the half-accumulator on DVE (idle during attention;
            # first in its FIFO region so the po slot frees promptly)
            nc.vector.tensor_copy(
                OTt[0:DH + 1, ihalf * 1024:(ihalf + 1) * 1024], po[:])

        def emit_tail_half(t, ihalf, OTt):
            # PE transpose (psum2) -> normalize (DVE) -> out DMA.  The
            # final head's outs go on the (empty) Sync HWDGE queue so the
            # kernel end isn't paced by GpSimd's slow SWDGE issue.
            out_eng = nc.sync if t == H_PER_CORE - 1 else nc.gpsimd
            for cb in range(ihalf * 8, ihalf * 8 + 8):
                ptr = psum2.tile([128, DH + 1], bf16, tag="ps2")
                nc.tensor.transpose(
                    ptr[:],
                    OTt[0:DH + 1, cb * 128:(cb + 1) * 128],
                    ident_bf[0:DH + 1, 0:DH + 1])
                recip = small_pool.tile([128, 1], f32, tag="recip")
                nc.vector.reciprocal(recip[:], ptr[:, DH:DH + 1])
                outt = small_pool.tile([128, DH], f32, tag="outt")
                nc.vector.tensor_scalar_mul(outt[:], ptr[:, 0:DH],
                                            recip[:])
                out_eng.dma_start(
                    out_dram.ap()[t * ROWS:(t + 1) * ROWS,
                                  cb * DH:(cb + 1) * DH],
                    outt[:])

        # ---- program order: later heads' projection/transposes deferred
        # so they fill PE gaps inside earlier ACT-bound attention ----
        OTs = [ot_pool.tile([DH + 1, SUB], bf16, tag="OTf", name=f"OTf{t}")
               for t in range(H_PER_CORE)]
        emit_proj(0)
        emit_transposes(0)
        emit_pass(0, 0, OTs[0])
        emit_proj(1)
        emit_transposes(1)
        emit_pass(0, 1, OTs[0])
        emit_tail_half(0, 0, OTs[0])
        emit_proj(2)
        emit_transposes(2)
        emit_pass(1, 0, OTs[1])
        emit_tail_half(0, 1, OTs[0])
        emit_proj(3)
        emit_transposes(3)
        emit_pass(1, 1, OTs[1])
        emit_tail_half(1, 0, OTs[1])
        emit_pass(2, 0, OTs[2])
        emit_tail_half(1, 1, OTs[1])
        emit_pass(2, 1, OTs[2])
        emit_tail_half(2, 0, OTs[2])
        emit_pass(3, 0, OTs[3])
        emit_tail_half(2, 1, OTs[2])
        emit_pass(3, 1, OTs[3])
        emit_tail_half(3, 0, OTs[3])
        emit_tail_half(3, 1, OTs[3])

    nc.compile()
    _GRAPH = nc
    return nc


def make_in_maps(x, w_qkv):
    w_bf = np.ascontiguousarray(w_qkv).astype(ml_dtypes.bfloat16)
    maps = []
    for c in range(N_CORES):
        b = c // 4
        r0 = (c % 4) * H_PER_CORE * ROWS
        xt = np.ascontiguousarray(
            x[b, r0:r0 + H_PER_CORE * ROWS, :].T).astype(ml_dtypes.bfloat16)
        maps.append({"xt": xt, "w": w_bf})
    return maps


def assemble_out(results):
    out = np.empty((B, N, D), dtype=np.float32)
    for c in range(N_CORES):
        b = c // 4
        r0 = (c % 4) * H_PER_CORE * ROWS
        out[b, r0:r0 + H_PER_CORE * ROWS, :] = results[c]["out"]
    return out


def kernel(x, w_qkv):
    from concourse import bass_utils
    nc = build_graph()
    res = bass_utils.run_bass_kernel_spmd(
        nc, make_in_maps(np.asarray(x), np.asarray(w_qkv)),
        list(range(N_CORES)))
    return assemble_out(res.results)



# revision 3
# speedup vs baseline: 1.0304x; 1.0304x over previous
"""Trainium2 Bass kernel for nn_Attention (dense transformer block-attention).

Reference semantics (faithful reshape WITHOUT head transpose):
  qkv = x @ w_qkv                    # [B, N, 3*1024]
  q = qkv[..., 0:1024].reshape(B, 16, 2048, 64)   # head h <- token rows [h*128,(h+1)*128)
  out[b, n, c] = O_head(n//128)[(n%128)*16 + c//64, c%64]

Sharding: 32 (b, head) pairs over 8 cores -> each core: 1 batch x 4 heads.
Pure data parallel, no collectives. Host preps xT (bf16) per core + full w (bf16).

v2 layout/schedule:
- Sub-token permutation n2' = cb*128 + r (softmax is permutation-invariant
  over keys; queries un-permuted via the output index mapping).
- qT holds the 64-wide head dim DUPLICATED on both partition halves; kT is
  stored as PAIRS: kTp[a][0:64] = k of key-block 2a, kTp[a][64:128] = block
  2a+1.  The S matmuls are ROW-TILED (tile_position (0,0)/(64,0)): two K=64
  matmuls run concurrently in the two halves of the PE array -> 2x S rate.
- PV: out^T = [v|ones].T @ exp(S^T): softmax denominators ride in row 64.
- Input DMA: 9 large transfers ordered by first consumption (q1,k1,v1,k2,
  v2,q2); head-0 attention starts after ~3MB instead of the full 7.3MB.
- Attention cycle order per head interleaves ic (query chunks) so the
  second key/value halves are only needed a few cycles in.
- Output assembled per head in SBUF, shipped as one contiguous 512KB DMA.
"""

import numpy as np
import ml_dtypes

B, N, D = 2, 2048, 1024
H_PER_CORE = 4          # head-blocks per core
ROWS = 128              # token rows per head-block
SUB = 2048              # sub-tokens per head (128 rows * 16 col-blocks)
DH = 64                 # head dim
CB = 16                 # col-blocks per row
SCALE = 0.125           # 64 ** -0.5
N_CORES = 8
KO = D // 128           # 8 k-tiles

_GRAPH = None


def build_graph():
    global _GRAPH
    if _GRAPH is not None:
        return _GRAPH

    import concourse.mybir as mybir
    import concourse.tile as tile
    from concourse import bacc
    from concourse.masks import make_identity
    from contextlib import ExitStack

    f32 = mybir.dt.float32
    bf16 = mybir.dt.bfloat16
    EXP = mybir.ActivationFunctionType.Exp

    nc = bacc.Bacc("TRN2", target_bir_lowering=False, debug=False,
                   num_devices=N_CORES)

    xt_dram = nc.dram_tensor("xt", [D, H_PER_CORE * ROWS], bf16,
                             kind="ExternalInput")
    w_dram = nc.dram_tensor("w", [D, 3 * D], bf16, kind="ExternalInput")
    out_dram = nc.dram_tensor("out", [H_PER_CORE * ROWS, D], f32,
                              kind="ExternalOutput")

    with tile.TileContext(nc) as tc, ExitStack() as ctx:
        const_pool = ctx.enter_context(tc.tile_pool(name="const", bufs=1))
        in_pool = ctx.enter_context(tc.tile_pool(name="inputs", bufs=1))
        qk_pool = ctx.enter_context(tc.tile_pool(name="qk", bufs=1))
        head_pool = ctx.enter_context(tc.tile_pool(name="head", bufs=1))
        pt_pool = ctx.enter_context(tc.tile_pool(name="pt", bufs=4))
        ot_pool = ctx.enter_context(tc.tile_pool(name="ot", bufs=1))
        ob_pool = ctx.enter_context(tc.tile_pool(name="ob", bufs=1))
        small_pool = ctx.enter_context(tc.tile_pool(name="small", bufs=16))
        ps_pool = ctx.enter_context(tc.tile_pool(name="ps", bufs=2,
                                                 space="PSUM"))
        psum2 = ctx.enter_context(tc.tile_pool(name="psum2", bufs=2,
                                               space="PSUM"))
        po_pool = ctx.enter_context(tc.tile_pool(name="po", bufs=2,
                                                 space="PSUM"))

        # ---- constants ----
        ident_bf = const_pool.tile([128, 128], bf16, tag="ident_bf")
        make_identity(nc, ident_bf[:])
        # warm up the exp table while the projection runs
        warm = const_pool.tile([128, 1], f32, tag="warm")
        nc.vector.memset(warm[:], 0.0)
        nc.scalar.activation(warm[:], warm[:], EXP)

        # ---- input DMA: large transfers in first-consumption order ----
        xt_sbuf = in_pool.tile([128, KO, H_PER_CORE * ROWS], bf16, tag="xt")
        w_sbuf = in_pool.tile([128, KO, 3 * D], bf16, tag="w")
        xt_src = xt_dram.ap().rearrange("(ko p) n -> p ko n", p=128)
        w_src = w_dram.ap().rearrange("(ko p) c -> p ko c", p=128)
        nc.sync.dma_start(xt_sbuf[:, :, :], xt_src)

        def w_load(c0, c1):
            nc.sync.dma_start(w_sbuf[:, :, c0:c1], w_src[:, :, c0:c1])

        w_load(0, 512)          # q cb 0-7    (ncx 0)
        w_load(1024, 1536)      # k cb 0-7    (ncx 2)
        w_load(2048, 2304)      # v cb 0-3    (ncx 4a)
        w_load(2304, 2560)      # v cb 4-7    (ncx 4b)
        w_load(1536, 2048)      # k cb 8-15   (ncx 3)
        w_load(2560, 2816)      # v cb 8-11   (ncx 5a)
        w_load(2816, 3072)      # v cb 12-15  (ncx 5b)
        w_load(512, 1024)       # q cb 8-15   (ncx 1)

        # ---- persistent per-head tiles ----
        qk2 = [qk_pool.tile([128, 2 * CB, 128], bf16, tag=f"qk2_{t}",
                            name=f"qk2_{t}")
               for t in range(H_PER_CORE)]
        qT = [head_pool.tile([128, SUB], bf16, tag=f"qT{t}", name=f"qT{t}")
              for t in range(H_PER_CORE)]
        # kTp[t][0:64, a, :] = k(block 2a), [64:128, a, :] = k(block 2a+1)
        kTp = [head_pool.tile([128, CB // 2, 128], bf16, tag=f"kTp{t}",
                              name=f"kTp{t}")
               for t in range(H_PER_CORE)]
        v_ones = [head_pool.tile([128, CB, DH + 1], bf16, tag=f"vo{t}",
                                 name=f"vo{t}")
                  for t in range(H_PER_CORE)]
        for t in range(H_PER_CORE):
            nc.vector.memset(v_ones[t][:, :, DH], 1.0)
        OTs = [ot_pool.tile([DH + 1, SUB], bf16, tag=f"OTf{t}",
                            name=f"OTf{t}")
               for t in range(H_PER_CORE)]
        outb = [ob_pool.tile([128, CB, DH], f32, tag=f"outb{t}",
                             name=f"outb{t}")
                for t in range(H_PER_CORE)]

        # ---- fill tasks (projection / transposes / tail) ----
        def task_proj(t, ncx, half=None):
            # half: None = full 512 cols; 0/1 = 256-col halves (for v, so
            # the task can start before the second half of its w DMA lands)
            if half is None:
                c0, w_cols = ncx * 512, 512
            else:
                c0, w_cols = ncx * 512 + half * 256, 256
            ps2 = psum2.tile([128, 512], f32, tag="ps2")
            for ko in range(KO):
                nc.tensor.matmul(
                    ps2[:, 0:w_cols],
                    xt_sbuf[:, ko, t * ROWS:(t + 1) * ROWS],
                    w_sbuf[:, ko, c0:c0 + w_cols],
                    start=(ko == 0), stop=(ko == KO - 1))
            src = ps2[:, 0:w_cols].rearrange("p (a b) -> p a b", b=DH)
            nblk = w_cols // DH
            b0 = (c0 % 1024) // DH
            if ncx < 2:        # q -> qk2 cb slots [b0, b0+nblk), duplicated
                nc.vector.tensor_copy(qk2[t][:, b0:b0 + nblk, 0:DH], src)
                nc.vector.tensor_copy(qk2[t][:, b0:b0 + nblk, DH:128], src)
            elif ncx < 4:      # k -> qk2 slots 16+cb, duplicated
                nc.vector.tensor_copy(
                    qk2[t][:, CB + b0:CB + b0 + nblk, 0:DH], src)
                nc.vector.tensor_copy(
                    qk2[t][:, CB + b0:CB + b0 + nblk, DH:128], src)
            else:              # v -> v_ones
                nc.vector.tensor_copy(
                    v_ones[t][:, b0:b0 + nblk, 0:DH], src)

        def task_tr_q(t, half):
            for i in range(8):
                cb = half * 8 + i
                pst = psum2.tile([128, 128], bf16, tag="ps2")
                nc.tensor.transpose(pst[:], qk2[t][:, cb, :], ident_bf[:])
                nc.vector.tensor_copy(
                    qT[t][:, cb * 128:(cb + 1) * 128], pst[:])

        def task_tr_k(t, half):
            # transpose of the DUPLICATED k chunk gives d on both halves;
            # take block 2a's d from the top half, 2a+1's from the bottom
            for i in range(8):
                cb = half * 8 + i
                a = cb // 2
                pst = psum2.tile([128, 128], bf16, tag="ps2")
                nc.tensor.transpose(pst[:], qk2[t][:, CB + cb, :],
                                    ident_bf[:])
                if cb % 2 == 0:
                    nc.vector.tensor_copy(kTp[t][0:64, a, :], pst[0:64, :])
                else:
                    nc.vector.tensor_copy(kTp[t][64:128, a, :],
                                          pst[64:128, :])

        def task_tail(t, half):
            for cb in range(half * 8, half * 8 + 8):
                ptr = psum2.tile([128, DH + 1], bf16, tag="ps2")
                nc.tensor.transpose(
                    ptr[:],
                    OTs[t][0:DH + 1, cb * 128:(cb + 1) * 128],
                    ident_bf[0:DH + 1, 0:DH + 1])
                recip = small_pool.tile([128, 1], f32, tag="recip")
                nc.vector.reciprocal(recip[:], ptr[:, DH:DH + 1])
                nc.vector.tensor_scalar_mul(outb[t][:, cb, :],
                                            ptr[:, 0:DH], recip[:])

        def task_outdma(t):
            nc.sync.dma_start(
                out_dram.ap()[t * ROWS:(t + 1) * ROWS, :]
                .rearrange("p (a b) -> p a b", b=DH),
                outb[t][:, :, :])

        # ---- attention atoms ----
        po_tiles = {}

        def S_pair(t, ic, a):
            ps = ps_pool.tile([128, 1024], f32, tag="ps")
            q0 = ic * 512
            nc.tensor.matmul(ps[:, 0:512],
                             kTp[t][0:64, a, :],
                             qT[t][0:64, q0:q0 + 512],
                             start=True, stop=True)
            nc.tensor.matmul(ps[:, 512:1024],
                             kTp[t][64:128, a, :],
                             qT[t][64:128, q0:q0 + 512],
                             start=True, stop=True)
            pt = pt_pool.tile([128, 1024], bf16, tag="pt")
            nc.scalar.activation(pt[:], ps[:], EXP, scale=SCALE)
            return pt

        def PV_pair(t, ic, a, pt):
            po = po_tiles[(t, ic)]
            nc.tensor.matmul(po[:], v_ones[t][:, 2 * a, :],
                             pt[:, 0:512],
                             start=(a == 0), stop=False,
                             skip_group_check=True)
            nc.tensor.matmul(po[:], v_ones[t][:, 2 * a + 1, :],
                             pt[:, 512:1024],
                             start=False, stop=(a == 7),
                             skip_group_check=True)

        # cycle c -> (ic, pair-group g): pairs (2g, 2g+1).  ic interleave
        # keeps the first 4 cycles on key-blocks 0-7 / v 0-7 so the second
        # k/v halves may still be in flight on the DMA.
        cycle_map = [(0, 0), (0, 1), (1, 0), (1, 1),
                     (0, 2), (0, 3), (1, 2), (1, 3),
                     (2, 0), (2, 1), (3, 0), (3, 1),
                     (2, 2), (2, 3), (3, 2), (3, 3)]

        def fill_sched(t):
            s = {}
            if t == 0:
                s[0] = [lambda: task_proj(0, 4, 0)]
                s[1] = [lambda: task_proj(0, 4, 1)]
                s[2] = [lambda: task_proj(0, 3)]
                s[3] = [lambda: task_tr_k(0, 1), lambda: task_proj(0, 5, 0)]
                s[4] = [lambda: task_proj(0, 5, 1)]
                s[6] = [lambda: task_proj(0, 1)]
                s[7] = [lambda: task_tr_q(0, 1)]
            else:
                s[0] = [lambda: task_proj(t, 3)]
                s[1] = [lambda: task_tr_k(t, 1), lambda: task_proj(t, 5, 0)]
                s[2] = [lambda: task_proj(t, 5, 1),
                        lambda: task_tail(t - 1, 1),
                        lambda: task_outdma(t - 1)]
                s[4] = [lambda: task_proj(t, 1)]
                s[5] = [lambda: task_tr_q(t, 1)]
            s[8] = [lambda: task_tail(t, 0)]
            if t < H_PER_CORE - 1:
                u = t + 1
                s[9] = [lambda: task_proj(u, 0)]
                s[10] = [lambda: task_proj(u, 2)]
                s[11] = [lambda: task_tr_q(u, 0)]
                s[12] = [lambda: task_tr_k(u, 0)]
                s[13] = [lambda: task_proj(u, 4, 0)]
                s[14] = [lambda: task_proj(u, 4, 1)]
            return s

        # ---- prologue: head 0 q1/k1 projection + transposes ----
        task_proj(0, 0)
        task_proj(0, 2)
        task_tr_q(0, 0)
        task_tr_k(0, 0)

        # ---- main loop ----
        for t in range(H_PER_CORE):
            sched = fill_sched(t)
            for c in range(16):
                ic, g = cycle_map[c]
                if g == 0:
                    po_tiles[(t, ic)] = po_pool.tile([DH + 1, 512], f32,
                                                     tag="po",
                                                     name=f"po_{t}_{ic}")
                pts = [S_pair(t, ic, 2 * g), S_pair(t, ic, 2 * g + 1)]
                for task in sched.get(c, []):
                    task()
                PV_pair(t, ic, 2 * g, pts[0])
                PV_pair(t, ic, 2 * g + 1, pts[1])
                if g == 3:
                    nc.vector.tensor_copy(
                        OTs[t][0:DH + 1, ic * 512:(ic + 1) * 512],
                        po_tiles[(t, ic)][:])

        # ---- epilogue ----
        task_tail(H_PER_CORE - 1, 1)
        task_outdma(H_PER_CORE - 1)

    nc.compile()
    _GRAPH = nc
    return nc


def make_in_maps(x, w_qkv):
    w_bf = np.ascontiguousarray(w_qkv).astype(ml_dtypes.bfloat16)
    maps = []
    for c in range(N_CORES):
        b = c // 4
        r0 = (c % 4) * H_PER_CORE * ROWS
        xt = np.ascontiguousarray(
            x[b, r0:r0 + H_PER_CORE * ROWS, :].T).astype(ml_dtypes.bfloat16)
        maps.append({"xt": xt, "w": w_bf})
    return maps


def assemble_out(results):
    out = np.empty((B, N, D), dtype=np.float32)
    for c in range(N_CORES):
        b = c // 4
        r0 = (c % 4) * H_PER_CORE * ROWS
        out[b, r0:r0 + H_PER_CORE * ROWS, :] = results[c]["out"]
    return out


def kernel(x, w_qkv):
    from concourse import bass_utils
    nc = build_graph()
    res = bass_utils.run_bass_kernel_spmd(
        nc, make_in_maps(np.asarray(x), np.asarray(w_qkv)),
        list(range(N_CORES)))
    return assemble_out(res.results)


# revision 8
# speedup vs baseline: 1.0717x; 1.0401x over previous
"""Trainium2 Bass kernel for nn_Attention (dense transformer block-attention).

Reference semantics (faithful reshape WITHOUT head transpose):
  qkv = x @ w_qkv                    # [B, N, 3*1024]
  q = qkv[..., 0:1024].reshape(B, 16, 2048, 64)   # head h <- token rows [h*128,(h+1)*128)
  out[b, n, c] = O_head(n//128)[(n%128)*16 + c//64, c%64]

Sharding: 32 (b, head) pairs over 8 cores -> each core: 1 batch x 4 heads.
Pure data parallel, no collectives. Host preps xT (bf16) per core + full w (bf16).

v3 design:
- Sub-token permutation n2' = cb*128 + r (softmax permutation-invariance).
- q/k projections are SWAPPED (w chunk stationary, xt streaming), so the
  projection output lands transposed:
  * k: ps[0:64,:] = d-vec of key-block 2wc, ps[64:128,:] = block 2wc+1 --
    exactly the row-tiled kTp stationary pair layout.  Zero transposes.
  * q: two constant selector matmuls ([I;I] stacked) turn the psum halves
    into the d-DUPLICATED qT stream layout.  Zero PE transposes.
- S matmuls are ROW-TILED (tile_position (0,0)/(64,0)): two K=64 matmuls
  run concurrently in the two PE halves -> 2x S rate.
- PV: out^T = [v|ones].T @ exp(S^T): softmax denominators ride in row 64.
- v projection unswapped (its natural output IS the PV stationary layout).
- Tail: 4 output-chunk transposes batched into ONE psum tile, then
  recip/mul into an SBUF-assembled output, shipped as 2 big DMAs per head.
- Input DMA: chunked, ordered by first consumption: xt, q(lo), k, v, q(hi).
"""

import numpy as np
import ml_dtypes

B, N, D = 2, 2048, 1024
H_PER_CORE = 4          # head-blocks per core
ROWS = 128              # token rows per head-block
SUB = 2048              # sub-tokens per head (128 rows * 16 col-blocks)
DH = 64                 # head dim
CB = 16                 # col-blocks per row
SCALE = 0.125           # 64 ** -0.5
N_CORES = 8
KO = D // 128           # 8 k-tiles

_GRAPH = None


def build_graph():
    global _GRAPH
    if _GRAPH is not None:
        return _GRAPH

    import concourse.mybir as mybir
    import concourse.tile as tile
    from concourse import bacc
    from concourse.masks import make_identity
    from contextlib import ExitStack

    f32 = mybir.dt.float32
    bf16 = mybir.dt.bfloat16
    EXP = mybir.ActivationFunctionType.Exp

    nc = bacc.Bacc("TRN2", target_bir_lowering=False, debug=False,
                   num_devices=N_CORES)

    xt_dram = nc.dram_tensor("xt", [D, H_PER_CORE * ROWS], bf16,
                             kind="ExternalInput")
    w_dram = nc.dram_tensor("w", [D, 3 * D], bf16, kind="ExternalInput")
    out_dram = nc.dram_tensor("out", [H_PER_CORE * ROWS, D], f32,
                              kind="ExternalOutput")

    with tile.TileContext(nc) as tc, ExitStack() as ctx:
        const_pool = ctx.enter_context(tc.tile_pool(name="const", bufs=1))
        in_pool = ctx.enter_context(tc.tile_pool(name="inputs", bufs=1))
        head_pool = ctx.enter_context(tc.tile_pool(name="head", bufs=1))
        pt_pool = ctx.enter_context(tc.tile_pool(name="pt", bufs=4))
        ot_pool = ctx.enter_context(tc.tile_pool(name="ot", bufs=1))
        ob_pool = ctx.enter_context(tc.tile_pool(name="ob", bufs=1))
        small_pool = ctx.enter_context(tc.tile_pool(name="small", bufs=16))
        ps_pool = ctx.enter_context(tc.tile_pool(name="ps", bufs=2,
                                                 space="PSUM"))
        psum2 = ctx.enter_context(tc.tile_pool(name="psum2", bufs=2,
                                               space="PSUM"))
        po_pool = ctx.enter_context(tc.tile_pool(name="po", bufs=2,
                                                 space="PSUM"))

        # ---- constants ----
        ident_bf = const_pool.tile([128, 128], bf16, tag="ident_bf")
        make_identity(nc, ident_bf[:])
        # selector matrices: sel_e = [[I64, I64], [0, 0]] picks psum rows
        # 0:64 and writes them (duplicated) to out rows 0:127; sel_o picks
        # rows 64:128.
        sel_e = const_pool.tile([128, 128], bf16, tag="sel_e")
        sel_o = const_pool.tile([128, 128], bf16, tag="sel_o")
        nc.vector.memset(sel_e[:], 0.0)
        nc.vector.memset(sel_o[:], 0.0)
        nc.vector.tensor_copy(sel_e[0:64, 0:64], ident_bf[0:64, 0:64])
        nc.vector.tensor_copy(sel_e[0:64, 64:128], ident_bf[0:64, 0:64])
        nc.vector.tensor_copy(sel_o[64:128, 0:64], ident_bf[64:128, 64:128])
        nc.vector.tensor_copy(sel_o[64:128, 64:128],
                              ident_bf[64:128, 64:128])
        # warm up the exp table while the prologue DMAs run
        warm = const_pool.tile([128, 1], f32, tag="warm")
        nc.vector.memset(warm[:], 0.0)
        nc.scalar.activation(warm[:], warm[:], EXP)

        # ---- input DMA: chunked, in first-consumption order ----
        xt_sbuf = in_pool.tile([128, KO, H_PER_CORE * ROWS], bf16, tag="xt")
        w_sbuf = in_pool.tile([128, KO, 3 * D], bf16, tag="w")
        xt_src = xt_dram.ap().rearrange("(ko p) n -> p ko n", p=128)
        w_src = w_dram.ap().rearrange("(ko p) c -> p ko c", p=128)
        nc.sync.dma_start(xt_sbuf[:, :, :], xt_src)

        def w_load(c0, c1):
            nc.sync.dma_start(w_sbuf[:, :, c0:c1], w_src[:, :, c0:c1])

        for wc in range(4):                 # q lo: cb 0-7
            w_load(wc * 128, (wc + 1) * 128)
        for wc in range(8):                 # k: all 16 blocks
            w_load(1024 + wc * 128, 1024 + (wc + 1) * 128)
        for h in range(4):                  # v: all 16 blocks
            w_load(2048 + h * 256, 2048 + (h + 1) * 256)
        for wc in range(4, 8):              # q hi: cb 8-15
            w_load(wc * 128, (wc + 1) * 128)

        # ---- persistent tiles ----
        # qTall[:, t, m]: duplicated-d q stream, m = cb*128 + r
        qTall = head_pool.tile([128, H_PER_CORE, SUB], bf16, tag="qTall")
        # qh[:, wc, 512]: swapped-q projection evac (cb parity in halves)
        qh = head_pool.tile([128, 8, 512], bf16, tag="qh")
        # kTp[0:64, t, a, :] = k(block 2a), [64:128, t, a, :] = k(2a+1)
        kTp = head_pool.tile([128, H_PER_CORE, CB // 2, 128], bf16,
                             tag="kTp")
        v_ones = [head_pool.tile([128, CB, DH + 1], bf16, tag=f"vo{t}",
                                 name=f"vo{t}")
                  for t in range(H_PER_CORE)]
        for t in range(H_PER_CORE):
            nc.vector.memset(v_ones[t][:, :, DH], 1.0)
        OTs = [ot_pool.tile([DH + 1, SUB], bf16, tag=f"OTf{t}",
                            name=f"OTf{t}")
               for t in range(H_PER_CORE)]
        outb = [ob_pool.tile([128, CB, DH], f32, tag=f"outb{t}",
                             name=f"outb{t}")
                for t in range(H_PER_CORE)]

        # ---- fill tasks ----
        def swk(wc, n0, n1):
            # swapped-k proj: kTp pair wc for token cols [n0,n1)
            ps2 = psum2.tile([128, n1 - n0], f32, tag="ps2",
                             name=f"k{wc}_{n0}")
            for ko in range(KO):
                nc.tensor.matmul(ps2[:],
                                 w_sbuf[:, ko, 1024 + wc * 128:
                                        1024 + (wc + 1) * 128],
                                 xt_sbuf[:, ko, n0:n1],
                                 start=(ko == 0), stop=(ko == KO - 1))
            t0, t1 = n0 // 128, n1 // 128
            nc.vector.tensor_copy(
                kTp[0:64, t0:t1, wc, :],
                ps2[0:64, :].rearrange("p (t r) -> p t r", r=128))
            nc.vector.tensor_copy(
                kTp[64:128, t0:t1, wc, :],
                ps2[64:128, :].rearrange("p (t r) -> p t r", r=128))

        def swq(wc, n0, n1):
            ps2 = psum2.tile([128, n1 - n0], f32, tag="ps2",
                             name=f"q{wc}_{n0}")
            for ko in range(KO):
                nc.tensor.matmul(ps2[:],
                                 w_sbuf[:, ko, wc * 128:(wc + 1) * 128],
                                 xt_sbuf[:, ko, n0:n1],
                                 start=(ko == 0), stop=(ko == KO - 1))
            nc.vector.tensor_copy(qh[:, wc, n0:n1], ps2[:])

        def sel(wc, n0, n1):
            t0, t1 = n0 // 128, n1 // 128
            for par, smat in ((0, sel_e), (1, sel_o)):
                ps2 = psum2.tile([128, n1 - n0], f32, tag="ps2",
                                 name=f"s{wc}_{par}_{n0}")
                nc.tensor.matmul(ps2[:], smat[:], qh[:, wc, n0:n1],
                                 start=True, stop=True)
                cb = 2 * wc + par
                nc.vector.tensor_copy(
                    qTall[:, t0:t1, cb * 128:(cb + 1) * 128],
                    ps2[:].rearrange("p (t r) -> p t r", r=128))

        def vproj(t, seg):
            # v col-blocks cb [4*seg, 4*seg+4) for head t
            c0 = 2048 + seg * 256
            ps2 = psum2.tile([128, 256], f32, tag="ps2", name=f"v{t}_{seg}")
            for ko in range(KO):
                nc.tensor.matmul(ps2[:],
                                 xt_sbuf[:, ko, t * ROWS:(t + 1) * ROWS],
                                 w_sbuf[:, ko, c0:c0 + 256],
                                 start=(ko == 0), stop=(ko == KO - 1))
            nc.vector.tensor_copy(
                v_ones[t][:, seg * 4:(seg + 1) * 4, 0:DH],
                ps2[:].rearrange("p (a b) -> p a b", b=DH))

        def tail4(t, qtr):
            # 4 output col-blocks: transposes batched into ONE psum tile
            # DH+2 stride keeps each block's PSUM offset 4-byte aligned
            ptr = psum2.tile([128, 4, DH + 2], bf16, tag="ps2",
                             name=f"t{t}_{qtr}")
            for i in range(4):
                cb = qtr * 4 + i
                nc.tensor.transpose(
                    ptr[:, i, 0:DH + 1],
                    OTs[t][0:DH + 1, cb * 128:(cb + 1) * 128],
                    ident_bf[0:DH + 1, 0:DH + 1])
            for i in range(4):
                cb = qtr * 4 + i
                recip = small_pool.tile([128, 1], f32, tag="recip")
                nc.vector.reciprocal(recip[:], ptr[:, i, DH:DH + 1])
                nc.vector.tensor_scalar_mul(outb[t][:, cb, :],
                                            ptr[:, i, 0:DH], recip[:])

        def outdma(t, half):
            nc.sync.dma_start(
                out_dram.ap()[t * ROWS:(t + 1) * ROWS,
                              half * 512:(half + 1) * 512]
                .rearrange("p (a b) -> p a b", b=DH),
                outb[t][:, half * 8:(half + 1) * 8, :])

        # ---- attention atoms ----
        po_tiles = {}

        def S_pair(t, ic, a):
            ps = ps_pool.tile([128, 1024], f32, tag="ps")
            q0 = ic * 512
            nc.tensor.matmul(ps[:, 0:512],
                             kTp[0:64, t, a, :],
                             qTall[0:64, t, q0:q0 + 512],
                             start=True, stop=True)
            nc.tensor.matmul(ps[:, 512:1024],
                             kTp[64:128, t, a, :],
                             qTall[64:128, t, q0:q0 + 512],
                             start=True, stop=True)
            pt = pt_pool.tile([128, 1024], bf16, tag="pt")
            nc.scalar.activation(pt[:], ps[:], EXP, scale=SCALE)
            return pt

        def PV_pair(t, ic, a, pt):
            po = po_tiles[(t, ic)]
            nc.tensor.matmul(po[:], v_ones[t][:, 2 * a, :],
                             pt[:, 0:512],
                             start=(a == 0), stop=False,
                             skip_group_check=True)
            nc.tensor.matmul(po[:], v_ones[t][:, 2 * a + 1, :],
                             pt[:, 512:1024],
                             start=False, stop=(a == 7),
                             skip_group_check=True)

        cycle_map = [(0, 0), (0, 1), (1, 0), (1, 1),
                     (0, 2), (0, 3), (1, 2), (1, 3),
                     (2, 0), (2, 1), (3, 0), (3, 1),
                     (2, 2), (2, 3), (3, 2), (3, 3)]

        # fill schedule: ("swq"/"swk", wc, n0, n1), ("sel", wc, n0, n1),
        # ("v", t, seg), ("tail", t, qtr), ("dma", t, half)
        SCHED = {
            0: {0: [("v", 0, 0)],
                1: [("v", 0, 1)],
                2: [("swk", 0, 128, 512), ("swk", 1, 128, 512)],
                3: [("v", 0, 2), ("swk", 2, 128, 512)],
                4: [("v", 0, 3), ("swk", 3, 128, 512)],
                5: [("swq", 4, 0, 128), ("sel", 4, 0, 128),
                    ("swk", 4, 128, 512)],
                6: [("swq", 5, 0, 128), ("sel", 5, 0, 128),
                    ("swk", 5, 128, 512)],
                7: [("swq", 6, 0, 128), ("sel", 6, 0, 128),
                    ("swk", 6, 128, 512)],
                8: [("tail", 0, 0), ("swq", 7, 0, 128),
                    ("sel", 7, 0, 128), ("swk", 7, 128, 512)],
                9: [("tail", 0, 1), ("dma", 0, 0)],
                10: [("swq", 0, 128, 512), ("sel", 0, 128, 512)],
                11: [("swq", 1, 128, 512), ("sel", 1, 128, 512)],
                12: [("v", 1, 0), ("swq", 2, 128, 512)],
                13: [("v", 1, 1), ("sel", 2, 128, 512)],
                14: [("v", 1, 2), ("swq", 3, 128, 512)],
                15: [("v", 1, 3), ("sel", 3, 128, 512)]},
            1: {0: [("swq", 4, 128, 512)],
                1: [("sel", 4, 128, 512)],
                2: [("tail", 0, 2)],
                3: [("tail", 0, 3), ("dma", 0, 1)],
                4: [("swq", 5, 128, 512)],
                5: [("sel", 5, 128, 512)],
                6: [("swq", 6, 128, 512)],
                7: [("sel", 6, 128, 512)],
                8: [("tail", 1, 0), ("swq", 7, 128, 512)],
                9: [("tail", 1, 1), ("dma", 1, 0), ("sel", 7, 128, 512)],
                12: [("v", 2, 0)], 13: [("v", 2, 1)],
                14: [("v", 2, 2)], 15: [("v", 2, 3)]},
            2: {2: [("tail", 1, 2)],
                3: [("tail", 1, 3), ("dma", 1, 1)],
                8: [("tail", 2, 0)],
                9: [("tail", 2, 1), ("dma", 2, 0)],
                12: [("v", 3, 0)], 13: [("v", 3, 1)],
                14: [("v", 3, 2)], 15: [("v", 3, 3)]},
            3: {2: [("tail", 2, 2)],
                3: [("tail", 2, 3), ("dma", 2, 1)],
                8: [("tail", 3, 0)],
                9: [("tail", 3, 1), ("dma", 3, 0)]},
        }
        TASK_FNS = {"swq": swq, "swk": swk, "sel": sel, "v": vproj,
                    "tail": tail4, "dma": outdma}

        # ---- prologue: head-0 slices of swapped q (lo) and k ----
        for wc in range(4):
            swq(wc, 0, 128)
            sel(wc, 0, 128)
        for wc in range(8):
            swk(wc, 0, 128)

        # ---- main loop ----
        for t in range(H_PER_CORE):
            for c in range(16):
                ic, g = cycle_map[c]
                if g == 0:
                    po_tiles[(t, ic)] = po_pool.tile([DH + 1, 512], f32,
                                                     tag="po",
                                                     name=f"po_{t}_{ic}")
                pts = [S_pair(t, ic, 2 * g), S_pair(t, ic, 2 * g + 1)]
                for task in SCHED[t].get(c, []):
                    TASK_FNS[task[0]](*task[1:])
                PV_pair(t, ic, 2 * g, pts[0])
                PV_pair(t, ic, 2 * g + 1, pts[1])
                if g == 3:
                    nc.vector.tensor_copy(
                        OTs[t][0:DH + 1, ic * 512:(ic + 1) * 512],
                        po_tiles[(t, ic)][:])

        # ---- epilogue ----
        t = H_PER_CORE - 1
        tail4(t, 2)
        tail4(t, 3)
        outdma(t, 1)

    nc.compile()
    _GRAPH = nc
    return nc


def make_in_maps(x, w_qkv):
    w_bf = np.ascontiguousarray(w_qkv).astype(ml_dtypes.bfloat16)
    maps = []
    for c in range(N_CORES):
        b = c // 4
        r0 = (c % 4) * H_PER_CORE * ROWS
        xt = np.ascontiguousarray(
            x[b, r0:r0 + H_PER_CORE * ROWS, :].T).astype(ml_dtypes.bfloat16)
        maps.append({"xt": xt, "w": w_bf})
    return maps


def assemble_out(results):
    out = np.empty((B, N, D), dtype=np.float32)
    for c in range(N_CORES):
        b = c // 4
        r0 = (c % 4) * H_PER_CORE * ROWS
        out[b, r0:r0 + H_PER_CORE * ROWS, :] = results[c]["out"]
    return out


def kernel(x, w_qkv):
    from concourse import bass_utils
    nc = build_graph()
    res = bass_utils.run_bass_kernel_spmd(
        nc, make_in_maps(np.asarray(x), np.asarray(w_qkv)),
        list(range(N_CORES)))
    return assemble_out(res.results)


# revision 9
# speedup vs baseline: 1.0745x; 1.0027x over previous
"""Trainium2 Bass kernel for nn_Attention (dense transformer block-attention).

Reference semantics (faithful reshape WITHOUT head transpose):
  qkv = x @ w_qkv                    # [B, N, 3*1024]
  q = qkv[..., 0:1024].reshape(B, 16, 2048, 64)   # head h <- token rows [h*128,(h+1)*128)
  out[b, n, c] = O_head(n//128)[(n%128)*16 + c//64, c%64]

Sharding: 32 (b, head) pairs over 8 cores -> each core: 1 batch x 4 heads.
Pure data parallel, no collectives. Host preps xT (bf16) per core + full w (bf16).

v3 design:
- Sub-token permutation n2' = cb*128 + r (softmax permutation-invariance).
- q/k projections are SWAPPED (w chunk stationary, xt streaming), so the
  projection output lands transposed:
  * k: ps[0:64,:] = d-vec of key-block 2wc, ps[64:128,:] = block 2wc+1 --
    exactly the row-tiled kTp stationary pair layout.  Zero transposes.
  * q: two constant selector matmuls ([I;I] stacked) turn the psum halves
    into the d-DUPLICATED qT stream layout.  Zero PE transposes.
- S matmuls are ROW-TILED (tile_position (0,0)/(64,0)): two K=64 matmuls
  run concurrently in the two PE halves -> 2x S rate.
- PV: out^T = [v|ones].T @ exp(S^T): softmax denominators ride in row 64.
- v projection unswapped (its natural output IS the PV stationary layout).
- Tail: 4 output-chunk transposes batched into ONE psum tile, then
  recip/mul into an SBUF-assembled output, shipped as 2 big DMAs per head.
- Input DMA: chunked, ordered by first consumption: xt, q(lo), k, v, q(hi).
"""

import numpy as np
import ml_dtypes

B, N, D = 2, 2048, 1024
H_PER_CORE = 4          # head-blocks per core
ROWS = 128              # token rows per head-block
SUB = 2048              # sub-tokens per head (128 rows * 16 col-blocks)
DH = 64                 # head dim
CB = 16                 # col-blocks per row
SCALE = 0.125           # 64 ** -0.5
N_CORES = 8
KO = D // 128           # 8 k-tiles

_GRAPH = None


def build_graph():
    global _GRAPH
    if _GRAPH is not None:
        return _GRAPH

    import concourse.mybir as mybir
    import concourse.tile as tile
    from concourse import bacc
    from concourse.masks import make_identity
    from contextlib import ExitStack

    f32 = mybir.dt.float32
    bf16 = mybir.dt.bfloat16
    EXP = mybir.ActivationFunctionType.Exp

    nc = bacc.Bacc("TRN2", target_bir_lowering=False, debug=False,
                   num_devices=N_CORES)

    xt_dram = nc.dram_tensor("xt", [D, H_PER_CORE * ROWS], bf16,
                             kind="ExternalInput")
    w_dram = nc.dram_tensor("w", [D, 3 * D], bf16, kind="ExternalInput")
    out_dram = nc.dram_tensor("out", [H_PER_CORE * ROWS, D], f32,
                              kind="ExternalOutput")

    with tile.TileContext(nc) as tc, ExitStack() as ctx:
        const_pool = ctx.enter_context(tc.tile_pool(name="const", bufs=1))
        in_pool = ctx.enter_context(tc.tile_pool(name="inputs", bufs=1))
        head_pool = ctx.enter_context(tc.tile_pool(name="head", bufs=1))
        pt_pool = ctx.enter_context(tc.tile_pool(name="pt", bufs=4))
        ot_pool = ctx.enter_context(tc.tile_pool(name="ot", bufs=1))
        ob_pool = ctx.enter_context(tc.tile_pool(name="ob", bufs=1))
        small_pool = ctx.enter_context(tc.tile_pool(name="small", bufs=4))
        ps_pool = ctx.enter_context(tc.tile_pool(name="ps", bufs=2,
                                                 space="PSUM"))
        psum2 = ctx.enter_context(tc.tile_pool(name="psum2", bufs=2,
                                               space="PSUM"))
        po_pool = ctx.enter_context(tc.tile_pool(name="po", bufs=2,
                                                 space="PSUM"))

        # ---- constants ----
        ident_bf = const_pool.tile([128, 128], bf16, tag="ident_bf")
        make_identity(nc, ident_bf[:])
        # selector matrices: sel_e = [[I64, I64], [0, 0]] picks psum rows
        # 0:64 and writes them (duplicated) to out rows 0:127; sel_o picks
        # rows 64:128.
        sel_e = const_pool.tile([128, 128], bf16, tag="sel_e")
        sel_o = const_pool.tile([128, 128], bf16, tag="sel_o")
        nc.vector.memset(sel_e[:], 0.0)
        nc.vector.memset(sel_o[:], 0.0)
        nc.vector.tensor_copy(sel_e[0:64, 0:64], ident_bf[0:64, 0:64])
        nc.vector.tensor_copy(sel_e[0:64, 64:128], ident_bf[0:64, 0:64])
        nc.vector.tensor_copy(sel_o[64:128, 0:64], ident_bf[64:128, 64:128])
        nc.vector.tensor_copy(sel_o[64:128, 64:128],
                              ident_bf[64:128, 64:128])
        # warm up the exp table while the prologue DMAs run
        warm = const_pool.tile([128, 1], f32, tag="warm")
        nc.vector.memset(warm[:], 0.0)
        nc.scalar.activation(warm[:], warm[:], EXP)

        # ---- input DMA: chunked, in first-consumption order ----
        xt_sbuf = in_pool.tile([128, KO, H_PER_CORE * ROWS], bf16, tag="xt")
        w_sbuf = in_pool.tile([128, KO, 3 * D], bf16, tag="w")
        xt_src = xt_dram.ap().rearrange("(ko p) n -> p ko n", p=128)
        w_src = w_dram.ap().rearrange("(ko p) c -> p ko c", p=128)
        nc.sync.dma_start(xt_sbuf[:, :, :], xt_src)

        def w_load(c0, c1):
            nc.sync.dma_start(w_sbuf[:, :, c0:c1], w_src[:, :, c0:c1])

        for wc in range(4):                 # q lo: cb 0-7
            w_load(wc * 128, (wc + 1) * 128)
        for wc in range(8):                 # k: all 16 blocks
            w_load(1024 + wc * 128, 1024 + (wc + 1) * 128)
        w_load(2048, 2560)                  # v lo (cb 0-7)
        w_load(2560, 3072)                  # v hi (cb 8-15)
        for wc in range(4, 8):              # q hi: cb 8-15
            w_load(wc * 128, (wc + 1) * 128)

        # ---- persistent tiles ----
        # qTall[:, t, m]: duplicated-d q stream, m = cb*128 + r
        qTall = head_pool.tile([128, H_PER_CORE, SUB], bf16, tag="qTall")
        # qh[:, wc, 512]: swapped-q projection evac (cb parity in halves)
        qh = head_pool.tile([128, 8, 512], bf16, tag="qh")
        # kTp[0:64, t, a, :] = k(block 2a), [64:128, t, a, :] = k(2a+1)
        kTp = head_pool.tile([128, H_PER_CORE, CB // 2, 128], bf16,
                             tag="kTp")
        v_ones = [head_pool.tile([128, CB, DH + 1], bf16, tag=f"vo{t}",
                                 name=f"vo{t}")
                  for t in range(H_PER_CORE)]
        for t in range(H_PER_CORE):
            nc.vector.memset(v_ones[t][:, :, DH], 1.0)
        OTs = [ot_pool.tile([DH + 1, SUB], bf16, tag=f"OTf{t}",
                            name=f"OTf{t}")
               for t in range(H_PER_CORE)]
        outb = [ob_pool.tile([128, CB, DH], f32, tag=f"outb{t}",
                             name=f"outb{t}")
                for t in range(H_PER_CORE)]

        # ---- fill tasks ----
        def swk(wc, n0, n1):
            # swapped-k proj: kTp pair wc for token cols [n0,n1)
            ps2 = psum2.tile([128, n1 - n0], f32, tag="ps2",
                             name=f"k{wc}_{n0}")
            for ko in range(KO):
                nc.tensor.matmul(ps2[:],
                                 w_sbuf[:, ko, 1024 + wc * 128:
                                        1024 + (wc + 1) * 128],
                                 xt_sbuf[:, ko, n0:n1],
                                 start=(ko == 0), stop=(ko == KO - 1))
            t0, t1 = n0 // 128, n1 // 128
            nc.vector.tensor_copy(
                kTp[0:64, t0:t1, wc, :],
                ps2[0:64, :].rearrange("p (t r) -> p t r", r=128))
            nc.vector.tensor_copy(
                kTp[64:128, t0:t1, wc, :],
                ps2[64:128, :].rearrange("p (t r) -> p t r", r=128))

        def swq(wc, n0, n1):
            ps2 = psum2.tile([128, n1 - n0], f32, tag="ps2",
                             name=f"q{wc}_{n0}")
            for ko in range(KO):
                nc.tensor.matmul(ps2[:],
                                 w_sbuf[:, ko, wc * 128:(wc + 1) * 128],
                                 xt_sbuf[:, ko, n0:n1],
                                 start=(ko == 0), stop=(ko == KO - 1))
            nc.vector.tensor_copy(qh[:, wc, n0:n1], ps2[:])

        def sel(wc, n0, n1):
            t0, t1 = n0 // 128, n1 // 128
            for par, smat in ((0, sel_e), (1, sel_o)):
                ps2 = psum2.tile([128, n1 - n0], f32, tag="ps2",
                                 name=f"s{wc}_{par}_{n0}")
                nc.tensor.matmul(ps2[:], smat[:], qh[:, wc, n0:n1],
                                 start=True, stop=True)
                cb = 2 * wc + par
                nc.vector.tensor_copy(
                    qTall[:, t0:t1, cb * 128:(cb + 1) * 128],
                    ps2[:].rearrange("p (t r) -> p t r", r=128))

        def vproj(t, seg):
            # v col-blocks cb [4*seg, 4*seg+4) for head t
            c0 = 2048 + seg * 256
            ps2 = psum2.tile([128, 256], f32, tag="ps2", name=f"v{t}_{seg}")
            for ko in range(KO):
                nc.tensor.matmul(ps2[:],
                                 xt_sbuf[:, ko, t * ROWS:(t + 1) * ROWS],
                                 w_sbuf[:, ko, c0:c0 + 256],
                                 start=(ko == 0), stop=(ko == KO - 1))
            nc.vector.tensor_copy(
                v_ones[t][:, seg * 4:(seg + 1) * 4, 0:DH],
                ps2[:].rearrange("p (a b) -> p a b", b=DH))

        def vhalf(t, half):
            # v col-blocks cb [8*half, 8*half+8) for head t (N=512 keeps
            # the per-ko LDWEIGHTS hidden under the stream)
            c0 = 2048 + half * 512
            ps2 = psum2.tile([128, 512], f32, tag="ps2", name=f"vh{t}_{half}")
            for ko in range(KO):
                nc.tensor.matmul(ps2[:],
                                 xt_sbuf[:, ko, t * ROWS:(t + 1) * ROWS],
                                 w_sbuf[:, ko, c0:c0 + 512],
                                 start=(ko == 0), stop=(ko == KO - 1))
            nc.vector.tensor_copy(
                v_ones[t][:, half * 8:(half + 1) * 8, 0:DH],
                ps2[:].rearrange("p (a b) -> p a b", b=DH))

        def tail4(t, qtr):
            # 4 output col-blocks: transposes batched into ONE psum tile
            # DH+2 stride keeps each block's PSUM offset 4-byte aligned
            ptr = psum2.tile([128, 4, DH + 2], bf16, tag="ps2",
                             name=f"t{t}_{qtr}")
            for i in range(4):
                cb = qtr * 4 + i
                nc.tensor.transpose(
                    ptr[:, i, 0:DH + 1],
                    OTs[t][0:DH + 1, cb * 128:(cb + 1) * 128],
                    ident_bf[0:DH + 1, 0:DH + 1])
            for i in range(4):
                cb = qtr * 4 + i
                recip = small_pool.tile([128, 1], f32, tag="recip")
                nc.vector.reciprocal(recip[:], ptr[:, i, DH:DH + 1])
                nc.vector.tensor_scalar_mul(outb[t][:, cb, :],
                                            ptr[:, i, 0:DH], recip[:])

        def outdma(t, half):
            nc.sync.dma_start(
                out_dram.ap()[t * ROWS:(t + 1) * ROWS,
                              half * 512:(half + 1) * 512]
                .rearrange("p (a b) -> p a b", b=DH),
                outb[t][:, half * 8:(half + 1) * 8, :])

        # ---- attention atoms ----
        po_tiles = {}

        def S_pair(t, ic, a):
            ps = ps_pool.tile([128, 1024], f32, tag="ps")
            q0 = ic * 512
            nc.tensor.matmul(ps[:, 0:512],
                             kTp[0:64, t, a, :],
                             qTall[0:64, t, q0:q0 + 512],
                             start=True, stop=True)
            nc.tensor.matmul(ps[:, 512:1024],
                             kTp[64:128, t, a, :],
                             qTall[64:128, t, q0:q0 + 512],
                             start=True, stop=True)
            pt = pt_pool.tile([128, 1024], bf16, tag="pt")
            nc.scalar.activation(pt[:], ps[:], EXP, scale=SCALE)
            return pt

        def PV_pair(t, ic, a, pt):
            po = po_tiles[(t, ic)]
            nc.tensor.matmul(po[:], v_ones[t][:, 2 * a, :],
                             pt[:, 0:512],
                             start=(a == 0), stop=False,
                             skip_group_check=True)
            nc.tensor.matmul(po[:], v_ones[t][:, 2 * a + 1, :],
                             pt[:, 512:1024],
                             start=False, stop=(a == 7),
                             skip_group_check=True)

        cycle_map = [(ic, g) for ic in range(4) for g in range(4)]

        # fill schedule: ("swq"/"swk"/"sel", wc, n0, n1), ("v", t, seg),
        # ("vh", t, half), ("tail", t, qtr), ("dma", t, half)
        SCHED = {
            0: {0: [("sel", 2, 0, 128), ("v", 0, 0), ("swk", 4, 0, 128)],
                1: [("sel", 3, 0, 128), ("v", 0, 1), ("swk", 5, 0, 128)],
                2: [("swk", 6, 0, 128), ("swk", 7, 0, 128), ("v", 0, 2)],
                3: [("v", 0, 3)],
                4: [("swq", 4, 0, 128), ("sel", 4, 0, 128)],
                5: [("swq", 5, 0, 128), ("sel", 5, 0, 128)],
                6: [("swq", 6, 0, 128), ("sel", 6, 0, 128)],
                7: [("swq", 7, 0, 128), ("sel", 7, 0, 128)],
                8: [("tail", 0, 0), ("swq", 0, 128, 512)],
                9: [("tail", 0, 1), ("dma", 0, 0), ("sel", 0, 128, 512)],
                10: [("swk", 0, 128, 512), ("swk", 1, 128, 512)],
                11: [("swk", 2, 128, 512), ("swq", 1, 128, 512)],
                12: [("swk", 3, 128, 512), ("sel", 1, 128, 512)],
                13: [("swk", 4, 128, 512), ("swk", 5, 128, 512)],
                14: [("swk", 6, 128, 512), ("swk", 7, 128, 512)],
                15: [("vh", 1, 0)]},
            1: {0: [("vh", 1, 1), ("swq", 2, 128, 512)],
                1: [("sel", 2, 128, 512), ("swq", 3, 128, 512)],
                2: [("sel", 3, 128, 512), ("tail", 0, 2)],
                3: [("tail", 0, 3), ("dma", 0, 1)],
                4: [("swq", 4, 128, 512)],
                5: [("sel", 4, 128, 512), ("swq", 5, 128, 512)],
                6: [("sel", 5, 128, 512)],
                7: [("swq", 6, 128, 512)],
                8: [("tail", 1, 0), ("sel", 6, 128, 512)],
                9: [("tail", 1, 1), ("dma", 1, 0), ("swq", 7, 128, 512)],
                10: [("sel", 7, 128, 512)],
                12: [("vh", 2, 0)],
                14: [("vh", 2, 1)]},
            2: {2: [("tail", 1, 2)],
                3: [("tail", 1, 3), ("dma", 1, 1)],
                8: [("tail", 2, 0)],
                9: [("tail", 2, 1), ("dma", 2, 0)],
                12: [("vh", 3, 0)],
                14: [("vh", 3, 1)]},
            3: {2: [("tail", 2, 2)],
                3: [("tail", 2, 3), ("dma", 2, 1)],
                8: [("tail", 3, 0)],
                9: [("tail", 3, 1), ("dma", 3, 0)]},
        }
        TASK_FNS = {"swq": swq, "swk": swk, "sel": sel, "v": vproj,
                    "vh": vhalf, "tail": tail4, "dma": outdma}

        # ---- prologue: head-0 slices of swapped q (lo) and k (lo) ----
        for wc in range(4):
            swq(wc, 0, 128)
        sel(0, 0, 128)
        sel(1, 0, 128)
        for wc in range(4):
            swk(wc, 0, 128)

        # ---- main loop ----
        for t in range(H_PER_CORE):
            for c in range(16):
                ic, g = cycle_map[c]
                if g == 0:
                    po_tiles[(t, ic)] = po_pool.tile([DH + 1, 512], f32,
                                                     tag="po",
                                                     name=f"po_{t}_{ic}")
                pts = [S_pair(t, ic, 2 * g), S_pair(t, ic, 2 * g + 1)]
                for task in SCHED[t].get(c, []):
                    TASK_FNS[task[0]](*task[1:])
                PV_pair(t, ic, 2 * g, pts[0])
                PV_pair(t, ic, 2 * g + 1, pts[1])
                if g == 3:
                    nc.vector.tensor_copy(
                        OTs[t][0:DH + 1, ic * 512:(ic + 1) * 512],
                        po_tiles[(t, ic)][:])

        # ---- epilogue ----
        t = H_PER_CORE - 1
        tail4(t, 2)
        tail4(t, 3)
        outdma(t, 1)

    nc.compile()
    _GRAPH = nc
    return nc


def make_in_maps(x, w_qkv):
    w_bf = np.ascontiguousarray(w_qkv).astype(ml_dtypes.bfloat16)
    maps = []
    for c in range(N_CORES):
        b = c // 4
        r0 = (c % 4) * H_PER_CORE * ROWS
        xt = np.ascontiguousarray(
            x[b, r0:r0 + H_PER_CORE * ROWS, :].T).astype(ml_dtypes.bfloat16)
        maps.append({"xt": xt, "w": w_bf})
    return maps


def assemble_out(results):
    out = np.empty((B, N, D), dtype=np.float32)
    for c in range(N_CORES):
        b = c // 4
        r0 = (c % 4) * H_PER_CORE * ROWS
        out[b, r0:r0 + H_PER_CORE * ROWS, :] = results[c]["out"]
    return out


def kernel(x, w_qkv):
    from concourse import bass_utils
    nc = build_graph()
    res = bass_utils.run_bass_kernel_spmd(
        nc, make_in_maps(np.asarray(x), np.asarray(w_qkv)),
        list(range(N_CORES)))
    return assemble_out(res.results)


# revision 10
# speedup vs baseline: 1.1363x; 1.0575x over previous
"""Trainium2 Bass kernel for nn_Attention (dense transformer block-attention).

Reference semantics (faithful reshape WITHOUT head transpose):
  qkv = x @ w_qkv                    # [B, N, 3*1024]
  q = qkv[..., 0:1024].reshape(B, 16, 2048, 64)   # head h <- token rows [h*128,(h+1)*128)
  out[b, n, c] = O_head(n//128)[(n%128)*16 + c//64, c%64]

Sharding: 32 (b, head) pairs over 8 cores -> each core: 1 batch x 4 heads.
Pure data parallel, no collectives. Host preps xT (bf16) per core + full w (bf16).

v3 design:
- Sub-token permutation n2' = cb*128 + r (softmax permutation-invariance).
- q/k projections are SWAPPED (w chunk stationary, xt streaming), so the
  projection output lands transposed:
  * k: ps[0:64,:] = d-vec of key-block 2wc, ps[64:128,:] = block 2wc+1 --
    exactly the row-tiled kTp stationary pair layout.  Zero transposes.
  * q: two constant selector matmuls ([I;I] stacked) turn the psum halves
    into the d-DUPLICATED qT stream layout.  Zero PE transposes.
- S matmuls are ROW-TILED (tile_position (0,0)/(64,0)): two K=64 matmuls
  run concurrently in the two PE halves -> 2x S rate.
- PV: out^T = [v|ones].T @ exp(S^T): softmax denominators ride in row 64.
- v projection unswapped (its natural output IS the PV stationary layout).
- Tail: 4 output-chunk transposes batched into ONE psum tile, then
  recip/mul into an SBUF-assembled output, shipped as 2 big DMAs per head.
- Input DMA: chunked, ordered by first consumption: xt, q(lo), k, v, q(hi).
"""

import numpy as np
import ml_dtypes

B, N, D = 2, 2048, 1024
H_PER_CORE = 4          # head-blocks per core
ROWS = 128              # token rows per head-block
SUB = 2048              # sub-tokens per head (128 rows * 16 col-blocks)
DH = 64                 # head dim
CB = 16                 # col-blocks per row
SCALE = 0.125           # 64 ** -0.5
N_CORES = 8
KO = D // 128           # 8 k-tiles

_GRAPH = None


def build_graph():
    global _GRAPH
    if _GRAPH is not None:
        return _GRAPH

    import concourse.mybir as mybir
    import concourse.tile as tile
    from concourse import bacc
    from concourse.masks import make_identity
    from contextlib import ExitStack

    f32 = mybir.dt.float32
    bf16 = mybir.dt.bfloat16
    EXP = mybir.ActivationFunctionType.Exp

    nc = bacc.Bacc("TRN2", target_bir_lowering=False, debug=False,
                   num_devices=N_CORES)

    xt_dram = nc.dram_tensor("xt", [D, H_PER_CORE * ROWS], bf16,
                             kind="ExternalInput")
    w_dram = nc.dram_tensor("w", [D, 3 * D], bf16, kind="ExternalInput")
    out_dram = nc.dram_tensor("out", [H_PER_CORE * ROWS, D], f32,
                              kind="ExternalOutput")

    with tile.TileContext(nc) as tc, ExitStack() as ctx:
        const_pool = ctx.enter_context(tc.tile_pool(name="const", bufs=1))
        in_pool = ctx.enter_context(tc.tile_pool(name="inputs", bufs=1))
        head_pool = ctx.enter_context(tc.tile_pool(name="head", bufs=1))
        pt_pool = ctx.enter_context(tc.tile_pool(name="pt", bufs=4))
        ot_pool = ctx.enter_context(tc.tile_pool(name="ot", bufs=1))
        ob_pool = ctx.enter_context(tc.tile_pool(name="ob", bufs=1))
        small_pool = ctx.enter_context(tc.tile_pool(name="small", bufs=4))
        ps_pool = ctx.enter_context(tc.tile_pool(name="ps", bufs=2,
                                                 space="PSUM"))
        psum2 = ctx.enter_context(tc.tile_pool(name="psum2", bufs=2,
                                               space="PSUM"))
        po_pool = ctx.enter_context(tc.tile_pool(name="po", bufs=2,
                                                 space="PSUM"))

        # ---- constants ----
        ident_bf = const_pool.tile([128, 128], bf16, tag="ident_bf")
        make_identity(nc, ident_bf[:])
        # selector matrices: sel_e = [[I64, I64], [0, 0]] picks psum rows
        # 0:64 and writes them (duplicated) to out rows 0:127; sel_o picks
        # rows 64:128.
        sel_e = const_pool.tile([128, 128], bf16, tag="sel_e")
        sel_o = const_pool.tile([128, 128], bf16, tag="sel_o")
        nc.vector.memset(sel_e[:], 0.0)
        nc.vector.memset(sel_o[:], 0.0)
        nc.vector.tensor_copy(sel_e[0:64, 0:64], ident_bf[0:64, 0:64])
        nc.vector.tensor_copy(sel_e[0:64, 64:128], ident_bf[0:64, 0:64])
        nc.vector.tensor_copy(sel_o[64:128, 0:64], ident_bf[64:128, 64:128])
        nc.vector.tensor_copy(sel_o[64:128, 64:128],
                              ident_bf[64:128, 64:128])
        # warm up the exp table while the prologue DMAs run
        warm = const_pool.tile([128, 1], f32, tag="warm")
        nc.vector.memset(warm[:], 0.0)
        nc.scalar.activation(warm[:], warm[:], EXP)

        # ---- input DMA: chunked, in first-consumption order ----
        xt_sbuf = in_pool.tile([128, KO, H_PER_CORE * ROWS], bf16, tag="xt")
        w_sbuf = in_pool.tile([128, KO, 3 * D], bf16, tag="w")
        xt_src = xt_dram.ap().rearrange("(ko p) n -> p ko n", p=128)
        w_src = w_dram.ap().rearrange("(ko p) c -> p ko c", p=128)
        nc.sync.dma_start(xt_sbuf[:, :, :], xt_src)

        def w_load(c0, c1):
            nc.sync.dma_start(w_sbuf[:, :, c0:c1], w_src[:, :, c0:c1])

        for wc in range(4):                 # q lo: cb 0-7
            w_load(wc * 128, (wc + 1) * 128)
        for wc in range(8):                 # k: all 16 blocks
            w_load(1024 + wc * 128, 1024 + (wc + 1) * 128)
        w_load(2048, 2560)                  # v lo (cb 0-7)
        w_load(2560, 3072)                  # v hi (cb 8-15)
        for wc in range(4, 8):              # q hi: cb 8-15
            w_load(wc * 128, (wc + 1) * 128)

        # ---- persistent tiles ----
        # qTall[:, t, m]: duplicated-d q stream, m = cb*128 + r
        qTall = head_pool.tile([128, H_PER_CORE, SUB], bf16, tag="qTall")
        # qh[:, wc, 512]: swapped-q projection evac (cb parity in halves)
        qh = head_pool.tile([128, 8, 512], bf16, tag="qh")
        # kTp[0:64, t, a, :] = k(block 2a), [64:128, t, a, :] = k(2a+1)
        kTp = head_pool.tile([128, H_PER_CORE, CB // 2, 128], bf16,
                             tag="kTp")
        v_ones = [head_pool.tile([128, CB, DH + 1], bf16, tag=f"vo{t}",
                                 name=f"vo{t}")
                  for t in range(H_PER_CORE)]
        for t in range(H_PER_CORE):
            nc.vector.memset(v_ones[t][:, :, DH], 1.0)
        OTs = [ot_pool.tile([DH + 1, SUB], bf16, tag=f"OTf{t}",
                            name=f"OTf{t}")
               for t in range(H_PER_CORE)]
        outb = [ob_pool.tile([128, CB, DH], f32, tag=f"outb{t}",
                             name=f"outb{t}")
                for t in range(H_PER_CORE)]

        # ---- fill tasks ----
        def swk(wc, n0, n1):
            # swapped-k proj: kTp pair wc for token cols [n0,n1)
            ps2 = psum2.tile([128, n1 - n0], f32, tag="ps2",
                             name=f"k{wc}_{n0}")
            for ko in range(KO):
                nc.tensor.matmul(ps2[:],
                                 w_sbuf[:, ko, 1024 + wc * 128:
                                        1024 + (wc + 1) * 128],
                                 xt_sbuf[:, ko, n0:n1],
                                 start=(ko == 0), stop=(ko == KO - 1))
            t0, t1 = n0 // 128, n1 // 128
            nc.vector.tensor_copy(
                kTp[0:64, t0:t1, wc, :],
                ps2[0:64, :].rearrange("p (t r) -> p t r", r=128))
            nc.vector.tensor_copy(
                kTp[64:128, t0:t1, wc, :],
                ps2[64:128, :].rearrange("p (t r) -> p t r", r=128))

        def swq(wc, n0, n1):
            ps2 = psum2.tile([128, n1 - n0], f32, tag="ps2",
                             name=f"q{wc}_{n0}")
            for ko in range(KO):
                nc.tensor.matmul(ps2[:],
                                 w_sbuf[:, ko, wc * 128:(wc + 1) * 128],
                                 xt_sbuf[:, ko, n0:n1],
                                 start=(ko == 0), stop=(ko == KO - 1))
            nc.vector.tensor_copy(qh[:, wc, n0:n1], ps2[:])

        def sel(wc, n0, n1):
            t0, t1 = n0 // 128, n1 // 128
            for par, smat in ((0, sel_e), (1, sel_o)):
                ps2 = psum2.tile([128, n1 - n0], f32, tag="ps2",
                                 name=f"s{wc}_{par}_{n0}")
                nc.tensor.matmul(ps2[:], smat[:], qh[:, wc, n0:n1],
                                 start=True, stop=True)
                cb = 2 * wc + par
                nc.vector.tensor_copy(
                    qTall[:, t0:t1, cb * 128:(cb + 1) * 128],
                    ps2[:].rearrange("p (t r) -> p t r", r=128))

        def vproj(t, seg):
            # v col-blocks cb [4*seg, 4*seg+4) for head t
            c0 = 2048 + seg * 256
            ps2 = psum2.tile([128, 256], f32, tag="ps2", name=f"v{t}_{seg}")
            for ko in range(KO):
                nc.tensor.matmul(ps2[:],
                                 xt_sbuf[:, ko, t * ROWS:(t + 1) * ROWS],
                                 w_sbuf[:, ko, c0:c0 + 256],
                                 start=(ko == 0), stop=(ko == KO - 1))
            nc.vector.tensor_copy(
                v_ones[t][:, seg * 4:(seg + 1) * 4, 0:DH],
                ps2[:].rearrange("p (a b) -> p a b", b=DH))

        def vhalf(t, half):
            # v col-blocks cb [8*half, 8*half+8) for head t (N=512 keeps
            # the per-ko LDWEIGHTS hidden under the stream)
            c0 = 2048 + half * 512
            ps2 = psum2.tile([128, 512], f32, tag="ps2", name=f"vh{t}_{half}")
            for ko in range(KO):
                nc.tensor.matmul(ps2[:],
                                 xt_sbuf[:, ko, t * ROWS:(t + 1) * ROWS],
                                 w_sbuf[:, ko, c0:c0 + 512],
                                 start=(ko == 0), stop=(ko == KO - 1))
            nc.vector.tensor_copy(
                v_ones[t][:, half * 8:(half + 1) * 8, 0:DH],
                ps2[:].rearrange("p (a b) -> p a b", b=DH))

        def tail4(t, qtr):
            # 4 output col-blocks: transposes batched into ONE psum tile
            # DH+2 stride keeps each block's PSUM offset 4-byte aligned
            ptr = psum2.tile([128, 4, DH + 2], bf16, tag="ps2",
                             name=f"t{t}_{qtr}")
            for i in range(4):
                cb = qtr * 4 + i
                nc.tensor.transpose(
                    ptr[:, i, 0:DH + 1],
                    OTs[t][0:DH + 1, cb * 128:(cb + 1) * 128],
                    ident_bf[0:DH + 1, 0:DH + 1])
            for i in range(4):
                cb = qtr * 4 + i
                recip = small_pool.tile([128, 1], f32, tag="recip")
                nc.vector.reciprocal(recip[:], ptr[:, i, DH:DH + 1])
                nc.vector.tensor_scalar_mul(outb[t][:, cb, :],
                                            ptr[:, i, 0:DH], recip[:])

        def outdma(t, half):
            nc.sync.dma_start(
                out_dram.ap()[t * ROWS:(t + 1) * ROWS,
                              half * 512:(half + 1) * 512]
                .rearrange("p (a b) -> p a b", b=DH),
                outb[t][:, half * 8:(half + 1) * 8, :])

        # ---- attention atoms ----
        po_tiles = {}

        def S_pair(t, ic, a):
            ps = ps_pool.tile([128, 1024], f32, tag="ps")
            q0 = ic * 512
            nc.tensor.matmul(ps[:, 0:512],
                             kTp[0:64, t, a, :],
                             qTall[0:64, t, q0:q0 + 512],
                             start=True, stop=True)
            nc.tensor.matmul(ps[:, 512:1024],
                             kTp[64:128, t, a, :],
                             qTall[64:128, t, q0:q0 + 512],
                             start=True, stop=True)
            pt = pt_pool.tile([128, 1024], bf16, tag="pt")
            nc.scalar.activation(pt[:], ps[:], EXP, scale=SCALE)
            return pt

        def PV_pair(t, ic, a, pt):
            po = po_tiles[(t, ic)]
            nc.tensor.matmul(po[:], v_ones[t][:, 2 * a, :],
                             pt[:, 0:512],
                             start=(a == 0), stop=False,
                             skip_group_check=True)
            nc.tensor.matmul(po[:], v_ones[t][:, 2 * a + 1, :],
                             pt[:, 512:1024],
                             start=False, stop=(a == 7),
                             skip_group_check=True)

        cycle_map = [(ic, g) for ic in range(4) for g in range(4)]

        # fill schedule: ("swq"/"swk"/"sel", wc, n0, n1), ("v", t, seg),
        # ("vh", t, half), ("tail", t, qtr), ("dma", t, half)
        SCHED = {
            0: {0: [("sel", 2, 0, 128), ("v", 0, 0)],
                1: [("sel", 3, 0, 128), ("v", 0, 1)],
                2: [("v", 0, 2)],
                3: [("v", 0, 3)],
                4: [("swq", 4, 0, 128), ("sel", 4, 0, 128)],
                5: [("swq", 5, 0, 128), ("sel", 5, 0, 128)],
                6: [("swq", 6, 0, 128), ("sel", 6, 0, 128)],
                7: [("swq", 7, 0, 128), ("sel", 7, 0, 128)],
                8: [("tail", 0, 0), ("swq", 0, 128, 512)],
                9: [("tail", 0, 1), ("dma", 0, 0), ("sel", 0, 128, 512)],
                10: [("swk", 0, 128, 512), ("swk", 1, 128, 512)],
                11: [("swk", 2, 128, 512), ("swq", 1, 128, 512)],
                12: [("swk", 3, 128, 512), ("sel", 1, 128, 512)],
                13: [("swk", 4, 128, 512), ("swk", 5, 128, 512)],
                14: [("swk", 6, 128, 512), ("swk", 7, 128, 512)],
                15: [("vh", 1, 0)]},
            1: {0: [("vh", 1, 1), ("swq", 2, 128, 512)],
                1: [("sel", 2, 128, 512), ("swq", 3, 128, 512)],
                2: [("sel", 3, 128, 512), ("tail", 0, 2)],
                3: [("tail", 0, 3), ("dma", 0, 1)],
                4: [("swq", 4, 128, 512)],
                5: [("sel", 4, 128, 512), ("swq", 5, 128, 512)],
                6: [("sel", 5, 128, 512)],
                7: [("swq", 6, 128, 512)],
                8: [("tail", 1, 0), ("sel", 6, 128, 512)],
                9: [("tail", 1, 1), ("dma", 1, 0), ("swq", 7, 128, 512)],
                10: [("sel", 7, 128, 512)],
                12: [("vh", 2, 0)],
                14: [("vh", 2, 1)]},
            2: {2: [("tail", 1, 2)],
                3: [("tail", 1, 3), ("dma", 1, 1)],
                8: [("tail", 2, 0)],
                9: [("tail", 2, 1), ("dma", 2, 0)],
                12: [("vh", 3, 0)],
                14: [("vh", 3, 1)]},
            3: {2: [("tail", 2, 2)],
                3: [("tail", 2, 3), ("dma", 2, 1)],
                8: [("tail", 3, 0)],
                9: [("tail", 3, 1), ("dma", 3, 0)]},
        }
        TASK_FNS = {"swq": swq, "swk": swk, "sel": sel, "v": vproj,
                    "vh": vhalf, "tail": tail4, "dma": outdma}

        # ---- prologue: head-0 slices of swapped q (lo) and k (lo) ----
        for wc in range(4):
            swq(wc, 0, 128)
        sel(0, 0, 128)
        sel(1, 0, 128)
        for wc in range(8):
            swk(wc, 0, 128)

        # ---- main loop ----
        slots = [(t, c) for t in range(H_PER_CORE) for c in range(16)]

        def emit_S(t, c):
            ic, g = cycle_map[c]
            if g == 0:
                po_tiles[(t, ic)] = po_pool.tile([DH + 1, 512], f32,
                                                 tag="po",
                                                 name=f"po_{t}_{ic}")
            return [S_pair(t, ic, 2 * g), S_pair(t, ic, 2 * g + 1)]

        pts = emit_S(*slots[0])
        for i, (t, c) in enumerate(slots):
            ic, g = cycle_map[c]
            nxt = emit_S(*slots[i + 1]) if i + 1 < len(slots) else None
            for task in SCHED[t].get(c, []):
                TASK_FNS[task[0]](*task[1:])
            PV_pair(t, ic, 2 * g, pts[0])
            PV_pair(t, ic, 2 * g + 1, pts[1])
            pts = nxt
            if g == 3:
                nc.vector.tensor_copy(
                    OTs[t][0:DH + 1, ic * 512:(ic + 1) * 512],
                    po_tiles[(t, ic)][:])

        # ---- epilogue ----
        t = H_PER_CORE - 1
        tail4(t, 2)
        tail4(t, 3)
        outdma(t, 1)

    nc.compile()
    _GRAPH = nc
    return nc


def make_in_maps(x, w_qkv):
    w_bf = np.ascontiguousarray(w_qkv).astype(ml_dtypes.bfloat16)
    maps = []
    for c in range(N_CORES):
        b = c // 4
        r0 = (c % 4) * H_PER_CORE * ROWS
        xt = np.ascontiguousarray(
            x[b, r0:r0 + H_PER_CORE * ROWS, :].T).astype(ml_dtypes.bfloat16)
        maps.append({"xt": xt, "w": w_bf})
    return maps


def assemble_out(results):
    out = np.empty((B, N, D), dtype=np.float32)
    for c in range(N_CORES):
        b = c // 4
        r0 = (c % 4) * H_PER_CORE * ROWS
        out[b, r0:r0 + H_PER_CORE * ROWS, :] = results[c]["out"]
    return out


def kernel(x, w_qkv):
    from concourse import bass_utils
    nc = build_graph()
    res = bass_utils.run_bass_kernel_spmd(
        nc, make_in_maps(np.asarray(x), np.asarray(w_qkv)),
        list(range(N_CORES)))
    return assemble_out(res.results)


# revision 11
# speedup vs baseline: 1.1916x; 1.0487x over previous
"""Trainium2 Bass kernel for nn_Attention (dense transformer block-attention).

Reference semantics (faithful reshape WITHOUT head transpose):
  qkv = x @ w_qkv                    # [B, N, 3*1024]
  q = qkv[..., 0:1024].reshape(B, 16, 2048, 64)   # head h <- token rows [h*128,(h+1)*128)
  out[b, n, c] = O_head(n//128)[(n%128)*16 + c//64, c%64]

Sharding: 32 (b, head) pairs over 8 cores -> each core: 1 batch x 4 heads.
Pure data parallel, no collectives. Host preps xT (bf16) per core + full w (bf16).

v3 design:
- Sub-token permutation n2' = cb*128 + r (softmax permutation-invariance).
- q/k projections are SWAPPED (w chunk stationary, xt streaming), so the
  projection output lands transposed:
  * k: ps[0:64,:] = d-vec of key-block 2wc, ps[64:128,:] = block 2wc+1 --
    exactly the row-tiled kTp stationary pair layout.  Zero transposes.
  * q: two constant selector matmuls ([I;I] stacked) turn the psum halves
    into the d-DUPLICATED qT stream layout.  Zero PE transposes.
- S matmuls are ROW-TILED (tile_position (0,0)/(64,0)): two K=64 matmuls
  run concurrently in the two PE halves -> 2x S rate.
- PV: out^T = [v|ones].T @ exp(S^T): softmax denominators ride in row 64.
- v projection unswapped (its natural output IS the PV stationary layout).
- Tail: 4 output-chunk transposes batched into ONE psum tile, then
  recip/mul into an SBUF-assembled output, shipped as 2 big DMAs per head.
- Input DMA: chunked, ordered by first consumption: xt, q(lo), k, v, q(hi).
"""

import numpy as np
import ml_dtypes

B, N, D = 2, 2048, 1024
H_PER_CORE = 4          # head-blocks per core
ROWS = 128              # token rows per head-block
SUB = 2048              # sub-tokens per head (128 rows * 16 col-blocks)
DH = 64                 # head dim
CB = 16                 # col-blocks per row
SCALE = 0.125           # 64 ** -0.5
N_CORES = 8
KO = D // 128           # 8 k-tiles

_GRAPH = None


def build_graph():
    global _GRAPH
    if _GRAPH is not None:
        return _GRAPH

    import concourse.mybir as mybir
    import concourse.tile as tile
    from concourse import bacc
    from concourse.masks import make_identity
    from contextlib import ExitStack

    f32 = mybir.dt.float32
    bf16 = mybir.dt.bfloat16
    EXP = mybir.ActivationFunctionType.Exp

    nc = bacc.Bacc("TRN2", target_bir_lowering=False, debug=False,
                   num_devices=N_CORES)

    xt_dram = nc.dram_tensor("xt", [D, H_PER_CORE * ROWS], bf16,
                             kind="ExternalInput")
    w_dram = nc.dram_tensor("w", [D, 3 * D], bf16, kind="ExternalInput")
    out_dram = nc.dram_tensor("out", [H_PER_CORE * ROWS, D], f32,
                              kind="ExternalOutput")

    with tile.TileContext(nc) as tc, ExitStack() as ctx:
        const_pool = ctx.enter_context(tc.tile_pool(name="const", bufs=1))
        in_pool = ctx.enter_context(tc.tile_pool(name="inputs", bufs=1))
        head_pool = ctx.enter_context(tc.tile_pool(name="head", bufs=1))
        pt_pool = ctx.enter_context(tc.tile_pool(name="pt", bufs=4))
        ot_pool = ctx.enter_context(tc.tile_pool(name="ot", bufs=1))
        ob_pool = ctx.enter_context(tc.tile_pool(name="ob", bufs=1))
        small_pool = ctx.enter_context(tc.tile_pool(name="small", bufs=4))
        ps_pool = ctx.enter_context(tc.tile_pool(name="ps", bufs=2,
                                                 space="PSUM"))
        psum2 = ctx.enter_context(tc.tile_pool(name="psum2", bufs=2,
                                               space="PSUM"))
        po_pool = ctx.enter_context(tc.tile_pool(name="po", bufs=2,
                                                 space="PSUM"))

        # ---- constants ----
        ident_bf = const_pool.tile([128, 128], bf16, tag="ident_bf")
        make_identity(nc, ident_bf[:])
        # selector matrices: sel_e = [[I64, I64], [0, 0]] picks psum rows
        # 0:64 and writes them (duplicated) to out rows 0:127; sel_o picks
        # rows 64:128.
        sel_e = const_pool.tile([128, 128], bf16, tag="sel_e")
        sel_o = const_pool.tile([128, 128], bf16, tag="sel_o")
        nc.vector.memset(sel_e[:], 0.0)
        nc.vector.memset(sel_o[:], 0.0)
        nc.vector.tensor_copy(sel_e[0:64, 0:64], ident_bf[0:64, 0:64])
        nc.vector.tensor_copy(sel_e[0:64, 64:128], ident_bf[0:64, 0:64])
        nc.vector.tensor_copy(sel_o[64:128, 0:64], ident_bf[64:128, 64:128])
        nc.vector.tensor_copy(sel_o[64:128, 64:128],
                              ident_bf[64:128, 64:128])
        # warm up the exp table while the prologue DMAs run
        warm = const_pool.tile([128, 1], f32, tag="warm")
        nc.vector.memset(warm[:], 0.0)
        nc.scalar.activation(warm[:], warm[:], EXP)

        # ---- input DMA: chunked, in first-consumption order ----
        xt_sbuf = in_pool.tile([128, KO, H_PER_CORE * ROWS], bf16, tag="xt")
        w_sbuf = in_pool.tile([128, KO, 3 * D], bf16, tag="w")
        xt_src = xt_dram.ap().rearrange("(ko p) n -> p ko n", p=128)
        w_src = w_dram.ap().rearrange("(ko p) c -> p ko c", p=128)
        nc.sync.dma_start(xt_sbuf[:, :, :], xt_src)

        def w_load(c0, c1):
            nc.sync.dma_start(w_sbuf[:, :, c0:c1], w_src[:, :, c0:c1])

        # Large 512-col loads: 1-2KB contiguous segments per (partition,ko)
        # keep DMA efficiency high; order = first consumption.
        w_load(0, 512)                      # q lo (cb 0-7)
        w_load(1024, 1536)                  # k lo (pairs 0-3)
        w_load(1536, 2048)                  # k hi (pairs 4-7)
        w_load(2048, 2560)                  # v lo (cb 0-7)
        w_load(2560, 3072)                  # v hi (cb 8-15)
        w_load(512, 1024)                   # q hi (cb 8-15)

        # ---- persistent tiles ----
        # qTall[:, t, m]: duplicated-d q stream, m = cb*128 + r
        qTall = head_pool.tile([128, H_PER_CORE, SUB], bf16, tag="qTall")
        # qh[:, wc, 512]: swapped-q projection evac (cb parity in halves)
        qh = head_pool.tile([128, 8, 512], bf16, tag="qh")
        # kTp[0:64, t, a, :] = k(block 2a), [64:128, t, a, :] = k(2a+1)
        kTp = head_pool.tile([128, H_PER_CORE, CB // 2, 128], bf16,
                             tag="kTp")
        v_ones = [head_pool.tile([128, CB, DH + 1], bf16, tag=f"vo{t}",
                                 name=f"vo{t}")
                  for t in range(H_PER_CORE)]
        for t in range(H_PER_CORE):
            nc.vector.memset(v_ones[t][:, :, DH], 1.0)
        OTs = [ot_pool.tile([DH + 1, SUB], bf16, tag=f"OTf{t}",
                            name=f"OTf{t}")
               for t in range(H_PER_CORE)]
        outb = [ob_pool.tile([128, CB, DH], f32, tag=f"outb{t}",
                             name=f"outb{t}")
                for t in range(H_PER_CORE)]

        # ---- fill tasks ----
        def swk(wc, n0, n1):
            # swapped-k proj: kTp pair wc for token cols [n0,n1)
            ps2 = psum2.tile([128, n1 - n0], f32, tag="ps2",
                             name=f"k{wc}_{n0}")
            for ko in range(KO):
                nc.tensor.matmul(ps2[:],
                                 w_sbuf[:, ko, 1024 + wc * 128:
                                        1024 + (wc + 1) * 128],
                                 xt_sbuf[:, ko, n0:n1],
                                 start=(ko == 0), stop=(ko == KO - 1))
            t0, t1 = n0 // 128, n1 // 128
            nc.vector.tensor_copy(
                kTp[0:64, t0:t1, wc, :],
                ps2[0:64, :].rearrange("p (t r) -> p t r", r=128))
            nc.vector.tensor_copy(
                kTp[64:128, t0:t1, wc, :],
                ps2[64:128, :].rearrange("p (t r) -> p t r", r=128))

        def swq(wc, n0, n1):
            ps2 = psum2.tile([128, n1 - n0], f32, tag="ps2",
                             name=f"q{wc}_{n0}")
            for ko in range(KO):
                nc.tensor.matmul(ps2[:],
                                 w_sbuf[:, ko, wc * 128:(wc + 1) * 128],
                                 xt_sbuf[:, ko, n0:n1],
                                 start=(ko == 0), stop=(ko == KO - 1))
            nc.vector.tensor_copy(qh[:, wc, n0:n1], ps2[:])

        def sel(wc, n0, n1):
            t0, t1 = n0 // 128, n1 // 128
            for par, smat in ((0, sel_e), (1, sel_o)):
                ps2 = psum2.tile([128, n1 - n0], f32, tag="ps2",
                                 name=f"s{wc}_{par}_{n0}")
                nc.tensor.matmul(ps2[:], smat[:], qh[:, wc, n0:n1],
                                 start=True, stop=True)
                cb = 2 * wc + par
                nc.vector.tensor_copy(
                    qTall[:, t0:t1, cb * 128:(cb + 1) * 128],
                    ps2[:].rearrange("p (t r) -> p t r", r=128))

        def vproj(t, seg):
            # v col-blocks cb [4*seg, 4*seg+4) for head t
            c0 = 2048 + seg * 256
            ps2 = psum2.tile([128, 256], f32, tag="ps2", name=f"v{t}_{seg}")
            for ko in range(KO):
                nc.tensor.matmul(ps2[:],
                                 xt_sbuf[:, ko, t * ROWS:(t + 1) * ROWS],
                                 w_sbuf[:, ko, c0:c0 + 256],
                                 start=(ko == 0), stop=(ko == KO - 1))
            nc.vector.tensor_copy(
                v_ones[t][:, seg * 4:(seg + 1) * 4, 0:DH],
                ps2[:].rearrange("p (a b) -> p a b", b=DH))

        def vhalf(t, half):
            # v col-blocks cb [8*half, 8*half+8) for head t (N=512 keeps
            # the per-ko LDWEIGHTS hidden under the stream)
            c0 = 2048 + half * 512
            ps2 = psum2.tile([128, 512], f32, tag="ps2", name=f"vh{t}_{half}")
            for ko in range(KO):
                nc.tensor.matmul(ps2[:],
                                 xt_sbuf[:, ko, t * ROWS:(t + 1) * ROWS],
                                 w_sbuf[:, ko, c0:c0 + 512],
                                 start=(ko == 0), stop=(ko == KO - 1))
            nc.vector.tensor_copy(
                v_ones[t][:, half * 8:(half + 1) * 8, 0:DH],
                ps2[:].rearrange("p (a b) -> p a b", b=DH))

        def tail4(t, qtr):
            # 4 output col-blocks: transposes batched into ONE psum tile
            # DH+2 stride keeps each block's PSUM offset 4-byte aligned
            ptr = psum2.tile([128, 4, DH + 2], bf16, tag="ps2",
                             name=f"t{t}_{qtr}")
            for i in range(4):
                cb = qtr * 4 + i
                nc.tensor.transpose(
                    ptr[:, i, 0:DH + 1],
                    OTs[t][0:DH + 1, cb * 128:(cb + 1) * 128],
                    ident_bf[0:DH + 1, 0:DH + 1])
            for i in range(4):
                cb = qtr * 4 + i
                recip = small_pool.tile([128, 1], f32, tag="recip")
                nc.vector.reciprocal(recip[:], ptr[:, i, DH:DH + 1])
                nc.vector.tensor_scalar_mul(outb[t][:, cb, :],
                                            ptr[:, i, 0:DH], recip[:])

        def outdma(t, half):
            nc.sync.dma_start(
                out_dram.ap()[t * ROWS:(t + 1) * ROWS,
                              half * 512:(half + 1) * 512]
                .rearrange("p (a b) -> p a b", b=DH),
                outb[t][:, half * 8:(half + 1) * 8, :])

        # ---- attention atoms ----
        po_tiles = {}

        def S_pair(t, ic, a):
            ps = ps_pool.tile([128, 1024], f32, tag="ps")
            q0 = ic * 512
            nc.tensor.matmul(ps[:, 0:512],
                             kTp[0:64, t, a, :],
                             qTall[0:64, t, q0:q0 + 512],
                             start=True, stop=True)
            nc.tensor.matmul(ps[:, 512:1024],
                             kTp[64:128, t, a, :],
                             qTall[64:128, t, q0:q0 + 512],
                             start=True, stop=True)
            pt = pt_pool.tile([128, 1024], bf16, tag="pt")
            nc.scalar.activation(pt[:], ps[:], EXP, scale=SCALE)
            return pt

        def PV_pair(t, ic, a, pt):
            po = po_tiles[(t, ic)]
            nc.tensor.matmul(po[:], v_ones[t][:, 2 * a, :],
                             pt[:, 0:512],
                             start=(a == 0), stop=False,
                             skip_group_check=True)
            nc.tensor.matmul(po[:], v_ones[t][:, 2 * a + 1, :],
                             pt[:, 512:1024],
                             start=False, stop=(a == 7),
                             skip_group_check=True)

        cycle_map = [(ic, g) for ic in range(4) for g in range(4)]

        # fill schedule: ("swq"/"swk"/"sel", wc, n0, n1), ("v", t, seg),
        # ("vh", t, half), ("tail", t, qtr), ("dma", t, half)
        SCHED = {
            0: {0: [("sel", 2, 0, 128), ("swk", 4, 0, 128),
                    ("swk", 5, 0, 128), ("v", 0, 0)],
                1: [("sel", 3, 0, 128), ("swk", 6, 0, 128),
                    ("swk", 7, 0, 128), ("v", 0, 1)],
                2: [("v", 0, 2)],
                3: [("v", 0, 3)],
                4: [("swq", 4, 0, 128), ("sel", 4, 0, 128)],
                5: [("swq", 5, 0, 128), ("sel", 5, 0, 128)],
                6: [("swq", 6, 0, 128), ("sel", 6, 0, 128)],
                7: [("swq", 7, 0, 128), ("sel", 7, 0, 128)],
                8: [("tail", 0, 0), ("swq", 0, 128, 512)],
                9: [("tail", 0, 1), ("dma", 0, 0), ("sel", 0, 128, 512)],
                10: [("swk", 0, 128, 512), ("swk", 1, 128, 512)],
                11: [("swk", 2, 128, 512), ("swq", 1, 128, 512)],
                12: [("swk", 3, 128, 512), ("sel", 1, 128, 512)],
                13: [("swk", 4, 128, 512), ("swk", 5, 128, 512)],
                14: [("swk", 6, 128, 512), ("swk", 7, 128, 512)],
                15: [("vh", 1, 0)]},
            1: {0: [("vh", 1, 1), ("swq", 2, 128, 512)],
                1: [("sel", 2, 128, 512), ("swq", 3, 128, 512)],
                2: [("sel", 3, 128, 512), ("tail", 0, 2)],
                3: [("tail", 0, 3), ("dma", 0, 1)],
                4: [("swq", 4, 128, 512)],
                5: [("sel", 4, 128, 512), ("swq", 5, 128, 512)],
                6: [("sel", 5, 128, 512)],
                7: [("swq", 6, 128, 512)],
                8: [("tail", 1, 0), ("sel", 6, 128, 512)],
                9: [("tail", 1, 1), ("dma", 1, 0), ("swq", 7, 128, 512)],
                10: [("sel", 7, 128, 512)],
                12: [("vh", 2, 0)],
                14: [("vh", 2, 1)]},
            2: {2: [("tail", 1, 2)],
                3: [("tail", 1, 3), ("dma", 1, 1)],
                8: [("tail", 2, 0)],
                9: [("tail", 2, 1), ("dma", 2, 0)],
                12: [("vh", 3, 0)],
                14: [("vh", 3, 1)]},
            3: {2: [("tail", 2, 2)],
                3: [("tail", 2, 3), ("dma", 2, 1)],
                8: [("tail", 3, 0)],
                9: [("tail", 3, 1), ("dma", 3, 0)]},
        }
        TASK_FNS = {"swq": swq, "swk": swk, "sel": sel, "v": vproj,
                    "vh": vhalf, "tail": tail4, "dma": outdma}

        # ---- prologue: head-0 slices of swapped q (lo) and k (lo) ----
        for wc in range(4):
            swq(wc, 0, 128)
        sel(0, 0, 128)
        sel(1, 0, 128)
        for wc in range(4):
            swk(wc, 0, 128)

        # ---- main loop ----
        slots = [(t, c) for t in range(H_PER_CORE) for c in range(16)]

        def emit_S(t, c):
            ic, g = cycle_map[c]
            if g == 0:
                po_tiles[(t, ic)] = po_pool.tile([DH + 1, 512], f32,
                                                 tag="po",
                                                 name=f"po_{t}_{ic}")
            return [S_pair(t, ic, 2 * g), S_pair(t, ic, 2 * g + 1)]

        pts = emit_S(*slots[0])
        for i, (t, c) in enumerate(slots):
            ic, g = cycle_map[c]
            nxt = emit_S(*slots[i + 1]) if i + 1 < len(slots) else None
            for task in SCHED[t].get(c, []):
                TASK_FNS[task[0]](*task[1:])
            PV_pair(t, ic, 2 * g, pts[0])
            PV_pair(t, ic, 2 * g + 1, pts[1])
            pts = nxt
            if g == 3:
                nc.vector.tensor_copy(
                    OTs[t][0:DH + 1, ic * 512:(ic + 1) * 512],
                    po_tiles[(t, ic)][:])

        # ---- epilogue ----
        t = H_PER_CORE - 1
        tail4(t, 2)
        tail4(t, 3)
        outdma(t, 1)

    nc.compile()
    _GRAPH = nc
    return nc


def make_in_maps(x, w_qkv):
    w_bf = np.ascontiguousarray(w_qkv).astype(ml_dtypes.bfloat16)
    maps = []
    for c in range(N_CORES):
        b = c // 4
        r0 = (c % 4) * H_PER_CORE * ROWS
        xt = np.ascontiguousarray(
            x[b, r0:r0 + H_PER_CORE * ROWS, :].T).astype(ml_dtypes.bfloat16)
        maps.append({"xt": xt, "w": w_bf})
    return maps


def assemble_out(results):
    out = np.empty((B, N, D), dtype=np.float32)
    for c in range(N_CORES):
        b = c // 4
        r0 = (c % 4) * H_PER_CORE * ROWS
        out[b, r0:r0 + H_PER_CORE * ROWS, :] = results[c]["out"]
    return out


def kernel(x, w_qkv):
    from concourse import bass_utils
    nc = build_graph()
    res = bass_utils.run_bass_kernel_spmd(
        nc, make_in_maps(np.asarray(x), np.asarray(w_qkv)),
        list(range(N_CORES)))
    return assemble_out(res.results)


# revision 13
# speedup vs baseline: 1.2083x; 1.0140x over previous
"""Trainium2 Bass kernel for nn_Attention (dense transformer block-attention).

Reference semantics (faithful reshape WITHOUT head transpose):
  qkv = x @ w_qkv                    # [B, N, 3*1024]
  q = qkv[..., 0:1024].reshape(B, 16, 2048, 64)   # head h <- token rows [h*128,(h+1)*128)
  out[b, n, c] = O_head(n//128)[(n%128)*16 + c//64, c%64]

Sharding: 32 (b, head) pairs over 8 cores -> each core: 1 batch x 4 heads.
Pure data parallel, no collectives. Host preps xT (bf16) per core + full w (bf16).

v3 design:
- Sub-token permutation n2' = cb*128 + r (softmax permutation-invariance).
- q/k projections are SWAPPED (w chunk stationary, xt streaming), so the
  projection output lands transposed:
  * k: ps[0:64,:] = d-vec of key-block 2wc, ps[64:128,:] = block 2wc+1 --
    exactly the row-tiled kTp stationary pair layout.  Zero transposes.
  * q: two constant selector matmuls ([I;I] stacked) turn the psum halves
    into the d-DUPLICATED qT stream layout.  Zero PE transposes.
- S matmuls are ROW-TILED (tile_position (0,0)/(64,0)): two K=64 matmuls
  run concurrently in the two PE halves -> 2x S rate.
- PV: out^T = [v|ones].T @ exp(S^T): softmax denominators ride in row 64.
- v projection unswapped (its natural output IS the PV stationary layout).
- Tail: 4 output-chunk transposes batched into ONE psum tile, then
  recip/mul into an SBUF-assembled output, shipped as 2 big DMAs per head.
- Input DMA: chunked, ordered by first consumption: xt, q(lo), k, v, q(hi).
"""

import numpy as np
import ml_dtypes

B, N, D = 2, 2048, 1024
H_PER_CORE = 4          # head-blocks per core
ROWS = 128              # token rows per head-block
SUB = 2048              # sub-tokens per head (128 rows * 16 col-blocks)
DH = 64                 # head dim
CB = 16                 # col-blocks per row
SCALE = 0.125           # 64 ** -0.5
N_CORES = 8
KO = D // 128           # 8 k-tiles

_GRAPH = None


def build_graph():
    global _GRAPH
    if _GRAPH is not None:
        return _GRAPH

    import concourse.mybir as mybir
    import concourse.tile as tile
    from concourse import bacc
    from concourse.masks import make_identity
    from contextlib import ExitStack

    f32 = mybir.dt.float32
    bf16 = mybir.dt.bfloat16
    EXP = mybir.ActivationFunctionType.Exp

    nc = bacc.Bacc("TRN2", target_bir_lowering=False, debug=False,
                   num_devices=N_CORES)

    xt_dram = nc.dram_tensor("xt", [D, H_PER_CORE * ROWS], bf16,
                             kind="ExternalInput")
    w_dram = nc.dram_tensor("w", [D, 3 * D], bf16, kind="ExternalInput")
    out_dram = nc.dram_tensor("out", [H_PER_CORE * ROWS, D], f32,
                              kind="ExternalOutput")

    with tile.TileContext(nc) as tc, ExitStack() as ctx:
        const_pool = ctx.enter_context(tc.tile_pool(name="const", bufs=1))
        in_pool = ctx.enter_context(tc.tile_pool(name="inputs", bufs=1))
        head_pool = ctx.enter_context(tc.tile_pool(name="head", bufs=1))
        pt_pool = ctx.enter_context(tc.tile_pool(name="pt", bufs=4))
        ot_pool = ctx.enter_context(tc.tile_pool(name="ot", bufs=1))
        ob_pool = ctx.enter_context(tc.tile_pool(name="ob", bufs=1))
        small_pool = ctx.enter_context(tc.tile_pool(name="small", bufs=4))
        ps_pool = ctx.enter_context(tc.tile_pool(name="ps", bufs=2,
                                                 space="PSUM"))
        psum2 = ctx.enter_context(tc.tile_pool(name="psum2", bufs=2,
                                               space="PSUM"))
        po_pool = ctx.enter_context(tc.tile_pool(name="po", bufs=2,
                                                 space="PSUM"))

        # ---- constants ----
        ident_bf = const_pool.tile([128, 128], bf16, tag="ident_bf")
        make_identity(nc, ident_bf[:])
        # selector matrices: sel_e = [[I64, I64], [0, 0]] picks psum rows
        # 0:64 and writes them (duplicated) to out rows 0:127; sel_o picks
        # rows 64:128.
        sel_e = const_pool.tile([128, 128], bf16, tag="sel_e")
        sel_o = const_pool.tile([128, 128], bf16, tag="sel_o")
        nc.vector.memset(sel_e[:], 0.0)
        nc.vector.memset(sel_o[:], 0.0)
        nc.vector.tensor_copy(sel_e[0:64, 0:64], ident_bf[0:64, 0:64])
        nc.vector.tensor_copy(sel_e[0:64, 64:128], ident_bf[0:64, 0:64])
        nc.vector.tensor_copy(sel_o[64:128, 0:64], ident_bf[64:128, 64:128])
        nc.vector.tensor_copy(sel_o[64:128, 64:128],
                              ident_bf[64:128, 64:128])
        # warm up the exp table while the prologue DMAs run
        warm = const_pool.tile([128, 1], f32, tag="warm")
        nc.vector.memset(warm[:], 0.0)
        nc.scalar.activation(warm[:], warm[:], EXP)

        # ---- input DMA: chunked, in first-consumption order ----
        xt_sbuf = in_pool.tile([128, KO, H_PER_CORE * ROWS], bf16, tag="xt")
        w_sbuf = in_pool.tile([128, KO, 3 * D], bf16, tag="w")
        xt_src = xt_dram.ap().rearrange("(ko p) n -> p ko n", p=128)
        w_src = w_dram.ap().rearrange("(ko p) c -> p ko c", p=128)
        nc.sync.dma_start(xt_sbuf[:, :, :], xt_src)

        def w_load(c0, c1):
            nc.sync.dma_start(w_sbuf[:, :, c0:c1], w_src[:, :, c0:c1])

        # Large 512-col loads: 1-2KB contiguous segments per (partition,ko)
        # keep DMA efficiency high; order = first consumption.
        w_load(0, 512)                      # q lo (cb 0-7)
        w_load(1024, 1536)                  # k lo (pairs 0-3)
        w_load(1536, 2048)                  # k hi (pairs 4-7)
        w_load(2048, 2560)                  # v lo (cb 0-7)
        w_load(2560, 3072)                  # v hi (cb 8-15)
        w_load(512, 1024)                   # q hi (cb 8-15)

        # ---- persistent tiles ----
        # qTall[:, t, m]: duplicated-d q stream, m = cb*128 + r
        qTall = head_pool.tile([128, H_PER_CORE, SUB], bf16, tag="qTall")
        # qh[:, wc, 512]: swapped-q projection evac (cb parity in halves)
        qh = head_pool.tile([128, 8, 512], bf16, tag="qh")
        # kTp[0:64, t, a, :] = k(block 2a), [64:128, t, a, :] = k(2a+1)
        kTp = head_pool.tile([128, H_PER_CORE, CB // 2, 128], bf16,
                             tag="kTp")
        v_ones = [head_pool.tile([128, CB, DH + 1], bf16, tag=f"vo{t}",
                                 name=f"vo{t}")
                  for t in range(H_PER_CORE)]
        for t in range(H_PER_CORE):
            nc.vector.memset(v_ones[t][:, :, DH], 1.0)
        OTs = [ot_pool.tile([DH + 1, SUB], bf16, tag=f"OTf{t}",
                            name=f"OTf{t}")
               for t in range(H_PER_CORE)]
        outb = [ob_pool.tile([128, CB, DH], f32, tag=f"outb{t}",
                             name=f"outb{t}")
                for t in range(H_PER_CORE)]

        # ---- fill tasks ----
        def swk(wc, n0, n1):
            # swapped-k proj: kTp pair wc for token cols [n0,n1)
            ps2 = psum2.tile([128, n1 - n0], f32, tag="ps2",
                             name=f"k{wc}_{n0}")
            for ko in range(KO):
                nc.tensor.matmul(ps2[:],
                                 w_sbuf[:, ko, 1024 + wc * 128:
                                        1024 + (wc + 1) * 128],
                                 xt_sbuf[:, ko, n0:n1],
                                 start=(ko == 0), stop=(ko == KO - 1))
            t0, t1 = n0 // 128, n1 // 128
            nc.vector.tensor_copy(
                kTp[0:64, t0:t1, wc, :],
                ps2[0:64, :].rearrange("p (t r) -> p t r", r=128))
            nc.vector.tensor_copy(
                kTp[64:128, t0:t1, wc, :],
                ps2[64:128, :].rearrange("p (t r) -> p t r", r=128))

        def swq(wc, n0, n1):
            ps2 = psum2.tile([128, n1 - n0], f32, tag="ps2",
                             name=f"q{wc}_{n0}")
            for ko in range(KO):
                nc.tensor.matmul(ps2[:],
                                 w_sbuf[:, ko, wc * 128:(wc + 1) * 128],
                                 xt_sbuf[:, ko, n0:n1],
                                 start=(ko == 0), stop=(ko == KO - 1))
            nc.vector.tensor_copy(qh[:, wc, n0:n1], ps2[:])

        def selm(par, wc0, nw, n0, n1):
            # one selector matmul covering nw w-chunks at once; the strided
            # evac scatters each chunk to its (odd/even) qTall column block
            t0, t1 = n0 // 128, n1 // 128
            nt = t1 - t0
            smat = sel_e if par == 0 else sel_o
            ps2 = psum2.tile([128, nw * (n1 - n0)], f32, tag="ps2",
                             name=f"s{par}_{wc0}_{n0}")
            nc.tensor.matmul(
                ps2[:], smat[:],
                qh[:, wc0:wc0 + nw, n0:n1],
                start=True, stop=True)
            dst = (qTall[:, t0:t1, :]
                   .rearrange("p t (c two r) -> p c two t r", two=2, r=128)
                   [:, wc0:wc0 + nw, par, :, :])
            nc.vector.tensor_copy(
                dst, ps2[:].rearrange("p (c t r) -> p c t r", t=nt, r=128))

        def vproj(t, seg):
            # v col-blocks cb [4*seg, 4*seg+4) for head t
            c0 = 2048 + seg * 256
            ps2 = psum2.tile([128, 256], f32, tag="ps2", name=f"v{t}_{seg}")
            for ko in range(KO):
                nc.tensor.matmul(ps2[:],
                                 xt_sbuf[:, ko, t * ROWS:(t + 1) * ROWS],
                                 w_sbuf[:, ko, c0:c0 + 256],
                                 start=(ko == 0), stop=(ko == KO - 1))
            nc.vector.tensor_copy(
                v_ones[t][:, seg * 4:(seg + 1) * 4, 0:DH],
                ps2[:].rearrange("p (a b) -> p a b", b=DH))

        def vhalf(t, half):
            # v col-blocks cb [8*half, 8*half+8) for head t (N=512 keeps
            # the per-ko LDWEIGHTS hidden under the stream)
            c0 = 2048 + half * 512
            ps2 = psum2.tile([128, 512], f32, tag="ps2", name=f"vh{t}_{half}")
            for ko in range(KO):
                nc.tensor.matmul(ps2[:],
                                 xt_sbuf[:, ko, t * ROWS:(t + 1) * ROWS],
                                 w_sbuf[:, ko, c0:c0 + 512],
                                 start=(ko == 0), stop=(ko == KO - 1))
            nc.vector.tensor_copy(
                v_ones[t][:, half * 8:(half + 1) * 8, 0:DH],
                ps2[:].rearrange("p (a b) -> p a b", b=DH))

        def tail4(t, qtr):
            # 4 output col-blocks: transposes batched into ONE psum tile
            # DH+2 stride keeps each block's PSUM offset 4-byte aligned
            ptr = psum2.tile([128, 4, DH + 2], bf16, tag="ps2",
                             name=f"t{t}_{qtr}")
            for i in range(4):
                cb = qtr * 4 + i
                nc.tensor.transpose(
                    ptr[:, i, 0:DH + 1],
                    OTs[t][0:DH + 1, cb * 128:(cb + 1) * 128],
                    ident_bf[0:DH + 1, 0:DH + 1])
            for i in range(4):
                cb = qtr * 4 + i
                recip = small_pool.tile([128, 1], f32, tag="recip")
                nc.vector.reciprocal(recip[:], ptr[:, i, DH:DH + 1])
                nc.vector.tensor_scalar_mul(outb[t][:, cb, :],
                                            ptr[:, i, 0:DH], recip[:])

        def outdma(t, half):
            nc.sync.dma_start(
                out_dram.ap()[t * ROWS:(t + 1) * ROWS,
                              half * 512:(half + 1) * 512]
                .rearrange("p (a b) -> p a b", b=DH),
                outb[t][:, half * 8:(half + 1) * 8, :])

        # ---- attention atoms ----
        po_tiles = {}

        def S_pair(t, ic, a):
            ps = ps_pool.tile([128, 1024], f32, tag="ps")
            q0 = ic * 512
            nc.tensor.matmul(ps[:, 0:512],
                             kTp[0:64, t, a, :],
                             qTall[0:64, t, q0:q0 + 512],
                             start=True, stop=True)
            nc.tensor.matmul(ps[:, 512:1024],
                             kTp[64:128, t, a, :],
                             qTall[64:128, t, q0:q0 + 512],
                             start=True, stop=True)
            pt = pt_pool.tile([128, 1024], bf16, tag="pt")
            nc.scalar.activation(pt[:], ps[:], EXP, scale=SCALE)
            return pt

        def PV_pair(t, ic, a, pt):
            po = po_tiles[(t, ic)]
            nc.tensor.matmul(po[:], v_ones[t][:, 2 * a, :],
                             pt[:, 0:512],
                             start=(a == 0), stop=False,
                             skip_group_check=True)
            nc.tensor.matmul(po[:], v_ones[t][:, 2 * a + 1, :],
                             pt[:, 512:1024],
                             start=False, stop=(a == 7),
                             skip_group_check=True)

        cycle_map = [(ic, g) for ic in range(4) for g in range(4)]

        # fill schedule: ("swq"/"swk"/"sel", wc, n0, n1), ("v", t, seg),
        # ("vh", t, half), ("tail", t, qtr), ("dma", t, half)
        SCHED = {
            0: {0: [("swk", 4, 0, 128), ("swk", 5, 0, 128), ("vh", 0, 0)],
                1: [("swk", 6, 0, 128), ("swk", 7, 0, 128)],
                2: [("vh", 0, 1)],
                3: [("vh", 1, 0)],
                4: [("swq", 4, 0, 128), ("swq", 5, 0, 128)],
                5: [("swq", 6, 0, 128), ("swq", 7, 0, 128)],
                6: [("selm", 0, 4, 4, 0, 128), ("selm", 1, 4, 4, 0, 128)],
                7: [],
                8: [("tail", 0, 0), ("swq", 0, 128, 512)],
                9: [("tail", 0, 1), ("dma", 0, 0), ("swq", 1, 128, 512)],
                10: [("swq", 2, 128, 512), ("swk", 0, 128, 512)],
                11: [("swq", 3, 128, 512), ("swk", 1, 128, 512)],
                12: [("swk", 2, 128, 512), ("swk", 3, 128, 512)],
                13: [("swk", 4, 128, 512), ("swk", 5, 128, 512),
                     ("selm", 0, 0, 1, 128, 512), ("selm", 1, 0, 1, 128, 512)],
                14: [("selm", 0, 1, 1, 128, 512), ("selm", 1, 1, 1, 128, 512),
                     ("selm", 0, 2, 1, 128, 512), ("swk", 6, 128, 512)],
                15: [("selm", 1, 2, 1, 128, 512), ("selm", 0, 3, 1, 128, 512),
                     ("selm", 1, 3, 1, 128, 512), ("swk", 7, 128, 512)]},
            1: {0: [("vh", 1, 1)],
                1: [("swq", 4, 128, 512)],
                2: [("swq", 5, 128, 512), ("tail", 0, 2)],
                3: [("tail", 0, 3), ("dma", 0, 1), ("swq", 6, 128, 512)],
                4: [("swq", 7, 128, 512)],
                5: [("selm", 0, 4, 1, 128, 512), ("selm", 1, 4, 1, 128, 512),
                    ("selm", 0, 5, 1, 128, 512), ("selm", 1, 5, 1, 128, 512)],
                6: [("selm", 0, 6, 1, 128, 512), ("selm", 1, 6, 1, 128, 512),
                    ("selm", 0, 7, 1, 128, 512), ("selm", 1, 7, 1, 128, 512)],
                8: [("tail", 1, 0)],
                9: [("tail", 1, 1), ("dma", 1, 0)],
                12: [("vh", 2, 0)],
                14: [("vh", 2, 1)]},
            2: {2: [("tail", 1, 2)],
                3: [("tail", 1, 3), ("dma", 1, 1)],
                8: [("tail", 2, 0)],
                9: [("tail", 2, 1), ("dma", 2, 0)],
                12: [("vh", 3, 0)],
                14: [("vh", 3, 1)]},
            3: {2: [("tail", 2, 2)],
                3: [("tail", 2, 3), ("dma", 2, 1)],
                8: [("tail", 3, 0)],
                9: [("tail", 3, 1), ("dma", 3, 0)]},
        }
        TASK_FNS = {"swq": swq, "swk": swk, "selm": selm, "v": vproj,
                    "vh": vhalf, "tail": tail4, "dma": outdma}

        # ---- prologue: head-0 slices of swapped q (lo) and k (lo) ----
        for wc in range(4):
            swq(wc, 0, 128)
        selm(0, 0, 4, 0, 128)
        selm(1, 0, 4, 0, 128)
        for wc in range(4):
            swk(wc, 0, 128)

        # ---- main loop ----
        slots = [(t, c) for t in range(H_PER_CORE) for c in range(16)]

        def emit_S(t, c):
            ic, g = cycle_map[c]
            if g == 0:
                po_tiles[(t, ic)] = po_pool.tile([DH + 1, 512], f32,
                                                 tag="po",
                                                 name=f"po_{t}_{ic}")
            return [S_pair(t, ic, 2 * g), S_pair(t, ic, 2 * g + 1)]

        pts = emit_S(*slots[0])
        for i, (t, c) in enumerate(slots):
            ic, g = cycle_map[c]
            nxt = emit_S(*slots[i + 1]) if i + 1 < len(slots) else None
            for task in SCHED[t].get(c, []):
                TASK_FNS[task[0]](*task[1:])
            PV_pair(t, ic, 2 * g, pts[0])
            PV_pair(t, ic, 2 * g + 1, pts[1])
            pts = nxt
            if g == 3:
                nc.vector.tensor_copy(
                    OTs[t][0:DH + 1, ic * 512:(ic + 1) * 512],
                    po_tiles[(t, ic)][:])

        # ---- epilogue ----
        t = H_PER_CORE - 1
        tail4(t, 2)
        tail4(t, 3)
        outdma(t, 1)

    nc.compile()
    _GRAPH = nc
    return nc


def make_in_maps(x, w_qkv):
    w_bf = np.ascontiguousarray(w_qkv).astype(ml_dtypes.bfloat16)
    maps = []
    for c in range(N_CORES):
        b = c // 4
        r0 = (c % 4) * H_PER_CORE * ROWS
        xt = np.ascontiguousarray(
            x[b, r0:r0 + H_PER_CORE * ROWS, :].T).astype(ml_dtypes.bfloat16)
        maps.append({"xt": xt, "w": w_bf})
    return maps


def assemble_out(results):
    out = np.empty((B, N, D), dtype=np.float32)
    for c in range(N_CORES):
        b = c // 4
        r0 = (c % 4) * H_PER_CORE * ROWS
        out[b, r0:r0 + H_PER_CORE * ROWS, :] = results[c]["out"]
    return out


def kernel(x, w_qkv):
    from concourse import bass_utils
    nc = build_graph()
    res = bass_utils.run_bass_kernel_spmd(
        nc, make_in_maps(np.asarray(x), np.asarray(w_qkv)),
        list(range(N_CORES)))
    return assemble_out(res.results)


# revision 15
# speedup vs baseline: 1.2225x; 1.0118x over previous
"""Trainium2 Bass kernel for nn_Attention (dense transformer block-attention).

Reference semantics (faithful reshape WITHOUT head transpose):
  qkv = x @ w_qkv                    # [B, N, 3*1024]
  q = qkv[..., 0:1024].reshape(B, 16, 2048, 64)   # head h <- token rows [h*128,(h+1)*128)
  out[b, n, c] = O_head(n//128)[(n%128)*16 + c//64, c%64]

Sharding: 32 (b, head) pairs over 8 cores -> each core: 1 batch x 4 heads.
Pure data parallel, no collectives. Host preps xT (bf16) per core + full w (bf16).

v3 design:
- Sub-token permutation n2' = cb*128 + r (softmax permutation-invariance).
- q/k projections are SWAPPED (w chunk stationary, xt streaming), so the
  projection output lands transposed:
  * k: ps[0:64,:] = d-vec of key-block 2wc, ps[64:128,:] = block 2wc+1 --
    exactly the row-tiled kTp stationary pair layout.  Zero transposes.
  * q: two constant selector matmuls ([I;I] stacked) turn the psum halves
    into the d-DUPLICATED qT stream layout.  Zero PE transposes.
- S matmuls are ROW-TILED (tile_position (0,0)/(64,0)): two K=64 matmuls
  run concurrently in the two PE halves -> 2x S rate.
- PV: out^T = [v|ones].T @ exp(S^T): softmax denominators ride in row 64.
- v projection unswapped (its natural output IS the PV stationary layout).
- Tail: 4 output-chunk transposes batched into ONE psum tile, then
  recip/mul into an SBUF-assembled output, shipped as 2 big DMAs per head.
- Input DMA: chunked, ordered by first consumption: xt, q(lo), k, v, q(hi).
"""

import numpy as np
import ml_dtypes

B, N, D = 2, 2048, 1024
H_PER_CORE = 4          # head-blocks per core
ROWS = 128              # token rows per head-block
SUB = 2048              # sub-tokens per head (128 rows * 16 col-blocks)
DH = 64                 # head dim
CB = 16                 # col-blocks per row
SCALE = 0.125           # 64 ** -0.5
N_CORES = 8
KO = D // 128           # 8 k-tiles

_GRAPH = None


def build_graph():
    global _GRAPH
    if _GRAPH is not None:
        return _GRAPH

    import concourse.mybir as mybir
    import concourse.tile as tile
    from concourse import bacc
    from concourse.masks import make_identity
    from contextlib import ExitStack

    f32 = mybir.dt.float32
    bf16 = mybir.dt.bfloat16
    EXP = mybir.ActivationFunctionType.Exp

    nc = bacc.Bacc("TRN2", target_bir_lowering=False, debug=False,
                   num_devices=N_CORES)

    xt_dram = nc.dram_tensor("xt", [D, H_PER_CORE * ROWS], bf16,
                             kind="ExternalInput")
    w_dram = nc.dram_tensor("w", [D, 3 * D], bf16, kind="ExternalInput")
    out_dram = nc.dram_tensor("out", [H_PER_CORE * ROWS, D], f32,
                              kind="ExternalOutput")

    with tile.TileContext(nc) as tc, ExitStack() as ctx:
        const_pool = ctx.enter_context(tc.tile_pool(name="const", bufs=1))
        in_pool = ctx.enter_context(tc.tile_pool(name="inputs", bufs=1))
        head_pool = ctx.enter_context(tc.tile_pool(name="head", bufs=1))
        pt_pool = ctx.enter_context(tc.tile_pool(name="pt", bufs=4))
        ot_pool = ctx.enter_context(tc.tile_pool(name="ot", bufs=1))
        ob_pool = ctx.enter_context(tc.tile_pool(name="ob", bufs=1))
        small_pool = ctx.enter_context(tc.tile_pool(name="small", bufs=4))
        ps_pool = ctx.enter_context(tc.tile_pool(name="ps", bufs=2,
                                                 space="PSUM"))
        psum2 = ctx.enter_context(tc.tile_pool(name="psum2", bufs=3,
                                               space="PSUM"))
        po_pool = ctx.enter_context(tc.tile_pool(name="po", bufs=1,
                                                 space="PSUM"))

        # ---- constants ----
        ident_bf = const_pool.tile([128, 128], bf16, tag="ident_bf")
        make_identity(nc, ident_bf[:])
        # selector matrices: sel_e = [[I64, I64], [0, 0]] picks psum rows
        # 0:64 and writes them (duplicated) to out rows 0:127; sel_o picks
        # rows 64:128.
        sel_e = const_pool.tile([128, 128], bf16, tag="sel_e")
        sel_o = const_pool.tile([128, 128], bf16, tag="sel_o")
        nc.vector.memset(sel_e[:], 0.0)
        nc.vector.memset(sel_o[:], 0.0)
        nc.vector.tensor_copy(sel_e[0:64, 0:64], ident_bf[0:64, 0:64])
        nc.vector.tensor_copy(sel_e[0:64, 64:128], ident_bf[0:64, 0:64])
        nc.vector.tensor_copy(sel_o[64:128, 0:64], ident_bf[64:128, 64:128])
        nc.vector.tensor_copy(sel_o[64:128, 64:128],
                              ident_bf[64:128, 64:128])
        # warm up the exp table while the prologue DMAs run
        warm = const_pool.tile([128, 1], f32, tag="warm")
        nc.vector.memset(warm[:], 0.0)
        nc.scalar.activation(warm[:], warm[:], EXP)

        # ---- input DMA: chunked, in first-consumption order ----
        xt_sbuf = in_pool.tile([128, KO, H_PER_CORE * ROWS], bf16, tag="xt")
        w_sbuf = in_pool.tile([128, KO, 3 * D], bf16, tag="w")
        xt_src = xt_dram.ap().rearrange("(ko p) n -> p ko n", p=128)
        w_src = w_dram.ap().rearrange("(ko p) c -> p ko c", p=128)
        nc.sync.dma_start(xt_sbuf[:, :, :], xt_src)

        def w_load(c0, c1):
            nc.sync.dma_start(w_sbuf[:, :, c0:c1], w_src[:, :, c0:c1])

        # Large 512-col loads: 1-2KB contiguous segments per (partition,ko)
        # keep DMA efficiency high; order = first consumption.
        w_load(0, 512)                      # q lo (cb 0-7)
        w_load(1024, 1536)                  # k lo (pairs 0-3)
        w_load(1536, 2048)                  # k hi (pairs 4-7)
        w_load(2048, 2560)                  # v lo (cb 0-7)
        w_load(2560, 3072)                  # v hi (cb 8-15)
        w_load(512, 1024)                   # q hi (cb 8-15)

        # ---- persistent tiles ----
        # qTall[:, t, m]: duplicated-d q stream, m = cb*128 + r
        qTall = head_pool.tile([128, H_PER_CORE, SUB], bf16, tag="qTall")
        # qh[:, wc, 512]: swapped-q projection evac (cb parity in halves)
        qh = head_pool.tile([128, 8, 512], bf16, tag="qh")
        # kTp[0:64, t, a, :] = k(block 2a), [64:128, t, a, :] = k(2a+1)
        kTp = head_pool.tile([128, H_PER_CORE, CB // 2, 128], bf16,
                             tag="kTp")
        v_ones = [head_pool.tile([128, CB, DH + 1], bf16, tag=f"vo{t}",
                                 name=f"vo{t}")
                  for t in range(H_PER_CORE)]
        for t in range(H_PER_CORE):
            nc.vector.memset(v_ones[t][:, :, DH], 1.0)
        OTs = [ot_pool.tile([DH + 1, SUB], bf16, tag=f"OTf{t}",
                            name=f"OTf{t}")
               for t in range(H_PER_CORE)]
        outb = [ob_pool.tile([128, CB, DH], f32, tag=f"outb{t}",
                             name=f"outb{t}")
                for t in range(H_PER_CORE)]

        # ---- fill tasks ----
        def swk(wc, n0, n1):
            # swapped-k proj: kTp pair wc for token cols [n0,n1)
            ps2 = psum2.tile([128, n1 - n0], f32, tag="ps2",
                             name=f"k{wc}_{n0}")
            for ko in range(KO):
                nc.tensor.matmul(ps2[:],
                                 w_sbuf[:, ko, 1024 + wc * 128:
                                        1024 + (wc + 1) * 128],
                                 xt_sbuf[:, ko, n0:n1],
                                 start=(ko == 0), stop=(ko == KO - 1))
            t0, t1 = n0 // 128, n1 // 128
            nc.vector.tensor_copy(
                kTp[0:64, t0:t1, wc, :],
                ps2[0:64, :].rearrange("p (t r) -> p t r", r=128))
            nc.vector.tensor_copy(
                kTp[64:128, t0:t1, wc, :],
                ps2[64:128, :].rearrange("p (t r) -> p t r", r=128))

        def swq(wc, n0, n1):
            ps2 = psum2.tile([128, n1 - n0], f32, tag="ps2",
                             name=f"q{wc}_{n0}")
            for ko in range(KO):
                nc.tensor.matmul(ps2[:],
                                 w_sbuf[:, ko, wc * 128:(wc + 1) * 128],
                                 xt_sbuf[:, ko, n0:n1],
                                 start=(ko == 0), stop=(ko == KO - 1))
            nc.vector.tensor_copy(qh[:, wc, n0:n1], ps2[:])

        def selm(par, wc0, nw, n0, n1):
            # one selector matmul covering nw w-chunks at once; the strided
            # evac scatters each chunk to its (odd/even) qTall column block
            t0, t1 = n0 // 128, n1 // 128
            nt = t1 - t0
            smat = sel_e if par == 0 else sel_o
            ps2 = psum2.tile([128, nw * (n1 - n0)], f32, tag="ps2",
                             name=f"s{par}_{wc0}_{n0}")
            nc.tensor.matmul(
                ps2[:], smat[:],
                qh[:, wc0:wc0 + nw, n0:n1],
                start=True, stop=True)
            dst = (qTall[:, t0:t1, :]
                   .rearrange("p t (c two r) -> p c two t r", two=2, r=128)
                   [:, wc0:wc0 + nw, par, :, :])
            nc.vector.tensor_copy(
                dst, ps2[:].rearrange("p (c t r) -> p c t r", t=nt, r=128))

        def qdma(wc0, nw, n0, n1):
            # build qTall for heads t0..t1 from qh via 4 SWDGE sbuf->sbuf
            # DMAs (partition shift + duplication) on the idle GpSimd queue
            t0, t1 = n0 // 128, n1 // 128
            nt = t1 - t0
            for par in (0, 1):
                srch = qh[par * 64:(par + 1) * 64, wc0:wc0 + nw, n0:n1]
                src = srch.rearrange("p c (t r) -> p c t r", r=128)
                for duph in (0, 1):
                    dst = (qTall[duph * 64:(duph + 1) * 64, t0:t1, :]
                           .rearrange("p t (c two r) -> p c two t r",
                                      two=2, r=128)
                           [:, wc0:wc0 + nw, par, :, :])
                    nc.gpsimd.dma_start(dst, src)

        def vproj(t, seg):
            # v col-blocks cb [4*seg, 4*seg+4) for head t
            c0 = 2048 + seg * 256
            ps2 = psum2.tile([128, 256], f32, tag="ps2", name=f"v{t}_{seg}")
            for ko in range(KO):
                nc.tensor.matmul(ps2[:],
                                 xt_sbuf[:, ko, t * ROWS:(t + 1) * ROWS],
                                 w_sbuf[:, ko, c0:c0 + 256],
                                 start=(ko == 0), stop=(ko == KO - 1))
            nc.vector.tensor_copy(
                v_ones[t][:, seg * 4:(seg + 1) * 4, 0:DH],
                ps2[:].rearrange("p (a b) -> p a b", b=DH))

        def vhalf(t, half):
            # v col-blocks cb [8*half, 8*half+8) for head t (N=512 keeps
            # the per-ko LDWEIGHTS hidden under the stream)
            c0 = 2048 + half * 512
            ps2 = psum2.tile([128, 512], f32, tag="ps2", name=f"vh{t}_{half}")
            for ko in range(KO):
                nc.tensor.matmul(ps2[:],
                                 xt_sbuf[:, ko, t * ROWS:(t + 1) * ROWS],
                                 w_sbuf[:, ko, c0:c0 + 512],
                                 start=(ko == 0), stop=(ko == KO - 1))
            nc.vector.tensor_copy(
                v_ones[t][:, half * 8:(half + 1) * 8, 0:DH],
                ps2[:].rearrange("p (a b) -> p a b", b=DH))

        def tail4(t, qtr):
            # 4 output col-blocks: transposes batched into ONE psum tile
            # DH+2 stride keeps each block's PSUM offset 4-byte aligned
            ptr = psum2.tile([128, 4, DH + 2], bf16, tag="ps2",
                             name=f"t{t}_{qtr}")
            for i in range(4):
                cb = qtr * 4 + i
                nc.tensor.transpose(
                    ptr[:, i, 0:DH + 1],
                    OTs[t][0:DH + 1, cb * 128:(cb + 1) * 128],
                    ident_bf[0:DH + 1, 0:DH + 1])
            for i in range(4):
                cb = qtr * 4 + i
                recip = small_pool.tile([128, 1], f32, tag="recip")
                nc.vector.reciprocal(recip[:], ptr[:, i, DH:DH + 1])
                nc.vector.tensor_scalar_mul(outb[t][:, cb, :],
                                            ptr[:, i, 0:DH], recip[:])

        def outdmaq(t, qtr):
            nc.sync.dma_start(
                out_dram.ap()[t * ROWS:(t + 1) * ROWS,
                              qtr * 256:(qtr + 1) * 256]
                .rearrange("p (a b) -> p a b", b=DH),
                outb[t][:, qtr * 4:(qtr + 1) * 4, :])

        def outdma(t, half):
            nc.sync.dma_start(
                out_dram.ap()[t * ROWS:(t + 1) * ROWS,
                              half * 512:(half + 1) * 512]
                .rearrange("p (a b) -> p a b", b=DH),
                outb[t][:, half * 8:(half + 1) * 8, :])

        # ---- attention atoms ----
        po_tiles = {}

        def S_pair(t, ic, a):
            ps = ps_pool.tile([128, 1024], f32, tag="ps")
            q0 = ic * 512
            nc.tensor.matmul(ps[:, 0:512],
                             kTp[0:64, t, a, :],
                             qTall[0:64, t, q0:q0 + 512],
                             start=True, stop=True)
            nc.tensor.matmul(ps[:, 512:1024],
                             kTp[64:128, t, a, :],
                             qTall[64:128, t, q0:q0 + 512],
                             start=True, stop=True)
            pt = pt_pool.tile([128, 1024], bf16, tag="pt")
            nc.scalar.activation(pt[:], ps[:], EXP, scale=SCALE)
            return pt

        def PV_pair(t, ic, a, pt):
            po = po_tiles[(t, ic)]
            nc.tensor.matmul(po[:], v_ones[t][:, 2 * a, :],
                             pt[:, 0:512],
                             start=(a == 0), stop=False,
                             skip_group_check=True)
            nc.tensor.matmul(po[:], v_ones[t][:, 2 * a + 1, :],
                             pt[:, 512:1024],
                             start=False, stop=(a == 7),
                             skip_group_check=True)

        cycle_map = [(ic, g) for ic in range(4) for g in range(4)]

        # fill schedule: ("swq"/"swk"/"sel", wc, n0, n1), ("v", t, seg),
        # ("vh", t, half), ("tail", t, qtr), ("dma", t, half)
        SCHED = {
            0: {0: [("swk", 4, 0, 128), ("swk", 5, 0, 128), ("vh", 0, 0)],
                1: [("swk", 6, 0, 128), ("swk", 7, 0, 128)],
                2: [("vh", 0, 1)],
                3: [("vh", 1, 0)],
                4: [("swq", 4, 0, 128), ("swq", 5, 0, 128)],
                5: [("swq", 6, 0, 128), ("swq", 7, 0, 128)],
                6: [("selm", 0, 4, 4, 0, 128), ("selm", 1, 4, 4, 0, 128)],
                7: [],
                8: [("tail", 0, 0), ("swq", 0, 128, 512)],
                9: [("tail", 0, 1), ("dma", 0, 0), ("swq", 1, 128, 512)],
                10: [("swq", 2, 128, 512), ("swk", 0, 128, 512)],
                11: [("swq", 3, 128, 512), ("swk", 1, 128, 512)],
                12: [("swk", 2, 128, 512), ("swk", 3, 128, 512)],
                13: [("swk", 4, 128, 512), ("swk", 5, 128, 512),
                     ("selm", 0, 0, 1, 128, 512), ("selm", 1, 0, 1, 128, 512)],
                14: [("selm", 0, 1, 1, 128, 512), ("selm", 1, 1, 1, 128, 512),
                     ("selm", 0, 2, 1, 128, 512), ("swk", 6, 128, 512)],
                15: [("selm", 1, 2, 1, 128, 512), ("selm", 0, 3, 1, 128, 512),
                     ("selm", 1, 3, 1, 128, 512), ("swk", 7, 128, 512)]},
            1: {0: [("vh", 1, 1)],
                1: [("swq", 4, 128, 512)],
                2: [("swq", 5, 128, 512), ("tail", 0, 2)],
                3: [("tail", 0, 3), ("dma", 0, 1), ("swq", 6, 128, 512)],
                4: [("swq", 7, 128, 512)],
                5: [("selm", 0, 4, 1, 128, 512), ("selm", 1, 4, 1, 128, 512),
                    ("selm", 0, 5, 1, 128, 512), ("selm", 1, 5, 1, 128, 512)],
                6: [("selm", 0, 6, 1, 128, 512), ("selm", 1, 6, 1, 128, 512),
                    ("selm", 0, 7, 1, 128, 512), ("selm", 1, 7, 1, 128, 512)],
                8: [("tail", 1, 0)],
                9: [("tail", 1, 1), ("dma", 1, 0)],
                12: [("vh", 2, 0)],
                14: [("vh", 2, 1)]},
            2: {2: [("tail", 1, 2)],
                3: [("tail", 1, 3), ("dma", 1, 1)],
                8: [("tail", 2, 0)],
                9: [("tail", 2, 1), ("dma", 2, 0)],
                12: [("vh", 3, 0)],
                14: [("vh", 3, 1)]},
            3: {2: [("tail", 2, 2)],
                3: [("tail", 2, 3), ("dma", 2, 1)],
                8: [("tail", 3, 0)],
                9: [("tail", 3, 1), ("dma", 3, 0)]},
        }
        TASK_FNS = {"swq": swq, "swk": swk, "selm": selm, "v": vproj,
                    "vh": vhalf, "tail": tail4, "dma": outdma,
                    "qdma": qdma}

        # ---- prologue: head-0 slices of swapped q (lo) and k (lo) ----
        for wc in range(4):
            swq(wc, 0, 128)
        selm(0, 0, 4, 0, 128)
        selm(1, 0, 4, 0, 128)
        for wc in range(4):
            swk(wc, 0, 128)

        # ---- main loop ----
        slots = [(t, c) for t in range(H_PER_CORE) for c in range(16)]

        def emit_S(t, c):
            ic, g = cycle_map[c]
            if g == 0:
                po_tiles[(t, ic)] = po_pool.tile([DH + 1, 512], f32,
                                                 tag="po",
                                                 name=f"po_{t}_{ic}")
            return [S_pair(t, ic, 2 * g), S_pair(t, ic, 2 * g + 1)]

        pts = emit_S(*slots[0])
        for i, (t, c) in enumerate(slots):
            ic, g = cycle_map[c]
            nxt = emit_S(*slots[i + 1]) if i + 1 < len(slots) else None
            for task in SCHED[t].get(c, []):
                TASK_FNS[task[0]](*task[1:])
            PV_pair(t, ic, 2 * g, pts[0])
            PV_pair(t, ic, 2 * g + 1, pts[1])
            pts = nxt
            if g == 3:
                nc.vector.tensor_copy(
                    OTs[t][0:DH + 1, ic * 512:(ic + 1) * 512],
                    po_tiles[(t, ic)][:])

        # ---- epilogue ----
        t = H_PER_CORE - 1
        tail4(t, 2)
        outdmaq(t, 2)
        tail4(t, 3)
        outdmaq(t, 3)

    nc.compile()
    _GRAPH = nc
    return nc


def make_in_maps(x, w_qkv):
    w_bf = np.ascontiguousarray(w_qkv).astype(ml_dtypes.bfloat16)
    maps = []
    for c in range(N_CORES):
        b = c // 4
        r0 = (c % 4) * H_PER_CORE * ROWS
        xt = np.ascontiguousarray(
            x[b, r0:r0 + H_PER_CORE * ROWS, :].T).astype(ml_dtypes.bfloat16)
        maps.append({"xt": xt, "w": w_bf})
    return maps


def assemble_out(results):
    out = np.empty((B, N, D), dtype=np.float32)
    for c in range(N_CORES):
        b = c // 4
        r0 = (c % 4) * H_PER_CORE * ROWS
        out[b, r0:r0 + H_PER_CORE * ROWS, :] = results[c]["out"]
    return out


def kernel(x, w_qkv):
    from concourse import bass_utils
    nc = build_graph()
    res = bass_utils.run_bass_kernel_spmd(
        nc, make_in_maps(np.asarray(x), np.asarray(w_qkv)),
        list(range(N_CORES)))
    return assemble_out(res.results)
